# revision 1
# baseline (speedup 1.0000x reference)
"""Trainium2 Bass kernel for nn_BaseRGCNHetero (3-layer heterogeneous RGCN).

Strategy (8 NeuronCores, SPMD):
  - Destination-shard the nodes: core c owns rows [c*N/8, (c+1)*N/8) of every
    node type; all edges whose dst is in the shard are processed there, so
    per-relation aggregates need no cross-core reduction.
  - Aggregate-first algebra: agg[dst] = (sum_{e->dst} h[src]) @ W_r * inv_deg,
    sharing one bf16 DRAM gather table per source ntype (drug, gene) per
    layer.  After each layer the drug/gene h-shards are AllGathered (bf16)
    into the next layer's tables; each AllGather is issued as soon as its
    ntype's windows finish so the wire time overlaps the remaining gathers.
  - Edge slot stream per relation: 128-dst windows in natural order, one
    chunk per (window, src bank), slots sorted by src row and padded to a
    multiple of 128 (pad slots point at an all-zero table row).  Non-
    transposed dma_gather pulls h[src] rows node-major ([slot, feat] blocks);
    gathers round-robin over 4 SWDGE queues so descriptor generation runs on
    all four GpSimd Q7 core pairs concurrently (3.3x single-queue, and the
    non-transpose path avoids the xbar that makes concurrent transposed
    gathers corrupt each other).
  - Segment sums on TensorE: per 128-slot block, VectorE builds a one-hot
    seg matrix seg[slot, dst] = (dstcol[slot] == dst) * inv_deg[slot] from
    host-streamed per-block columns; matmul(gt_block^T @ seg) accumulates
    agg[feat, dst] for the window in PSUM across the window's blocks.
  - Per dst window: agg -> bf16 praw, then one matmul per relation (W_r) plus
    the self-loop h @ L accumulate in a single PSUM bank; bias (+relu) is a
    fused ScalarE activation per window.
"""
import sys
import types
import numpy as np
import ml_dtypes
from contextlib import ExitStack

import concourse.bass as bass
import concourse.bacc as bacc
import concourse.tile as tile
from concourse import mybir, library_config

BF16 = ml_dtypes.bfloat16
P = 128
NQ = 4             # SWDGE queues (gather descriptor-gen parallelism)

CFG = dict(
    N={"drug": 20000, "gene": 50000, "disease": 10000},
    MOD={"drug": 1024, "gene": 768, "disease": 512},
    D_IN=128, D_H=128, D_OUT=64,
    RELS=[("drug", "disease", "dd"), ("drug", "drug", "ddr"),
          ("drug", "gene", "dg"), ("gene", "disease", "gd"),
          ("gene", "gene", "gg")],
    NCORE=8,
    BANK=32768,     # dma_gather int16 row-index limit per table slice
)

NTYPES = ("drug", "gene", "disease")
SRC_NTYPES = ("drug", "gene")
# layer processing phases: dst ntype -> relations feeding it (tags).
# drug first: its AllGather (issued right after the drug windows) then hides
# under the gene+disease phases; the gene AllGather hides under disease.
PHASES = [("drug", ["ddr"]), ("gene", ["dg", "gg"]), ("disease", ["dd", "gd"])]


# ---------------------------------------------------------------------------
# host-side preprocessing
# ---------------------------------------------------------------------------

def _pack_idx(stream):
    """int array (len % 128 == 0) -> dma_gather idx layout [128, len/16] int16:
    idx i at (i%16, i//16), replicated across the 8 groups of 16 partitions."""
    n = stream.size
    v = stream.astype(np.int16).reshape(n // 16, 16).T
    return np.tile(v, (8, 1))


def _banks(cfg, snt):
    """Gather-table bank slices for source ntype snt.
    Table rows: 0 = zeros, 1..N = nodes, N+1 = zeros.
    Returns list of (start_row, end_row, pad_row_relative)."""
    n = cfg["N"][snt]
    trows = n + 2
    if trows <= cfg["BANK"]:
        return [(0, trows, 0)]
    return [(0, cfg["BANK"], 0), (cfg["BANK"], trows, n + 1 - cfg["BANK"])]


def preprocess(cfg, inputs):
    ncore = cfg["NCORE"]
    shard = {nt: cfg["N"][nt] // ncore for nt in NTYPES}
    nw = {nt: -(-shard[nt] // P) for nt in NTYPES}

    S = dict(cfg=cfg, nw=nw, shard=shard, rels=[])
    percore = [dict() for _ in range(ncore)]

    for r, (snt, dnt, tag) in enumerate(cfg["RELS"]):
        src = np.asarray(inputs["e_" + tag + "_s"]).astype(np.int64)
        dst = np.asarray(inputs["e_" + tag + "_d"]).astype(np.int64)
        banks = _banks(cfg, snt)
        nbank = len(banks)
        NW = nw[dnt]
        dsh = shard[dnt]

        core_of = dst // dsh
        deg_all = np.bincount(dst, minlength=cfg["N"][dnt]).astype(np.float32)
        inv_deg = 1.0 / np.maximum(deg_all, 1.0)

        row_all = src + 1
        bank_of = (row_all >= cfg["BANK"]).astype(np.int64) if nbank == 2 \
            else np.zeros(row_all.size, np.int64)

        # per-core per-(window, bank) edge counts -> shared block counts
        cnt = np.zeros((ncore, NW, nbank), np.int64)
        ld_all = dst - core_of * dsh
        w_all = ld_all // P
        for c in range(ncore):
            m = core_of == c
            key = w_all[m] * nbank + bank_of[m]
            cnt[c] = np.bincount(key, minlength=NW * nbank).reshape(NW, nbank)
        nblk = -(-cnt.max(axis=0) // P)          # [NW, nbank] shared
        slots_wb = nblk * P
        off_wb = np.zeros((NW, nbank), np.int64)
        gathers = []                              # (bank, off, slots, nblk, segoff)
        off = 0
        segoff = 0
        for w in range(NW):
            for b in range(nbank):
                if nblk[w, b] == 0:
                    continue
                off_wb[w, b] = off
                gathers.append((w, b, int(off), int(slots_wb[w, b]),
                                int(nblk[w, b]), int(segoff)))
                off += int(slots_wb[w, b])
                segoff += int(nblk[w, b])
        nslots = max(off, P)
        nblk_tot = max(segoff, 1)
        maxblk = int(nblk.max()) if nblk.size else 1

        for c in range(ncore):
            stream = np.zeros(nslots, np.int16)
            segm = np.zeros((nblk_tot, P, P), np.float32)
            for w, b, o, sl, nb, so in gathers:
                stream[o:o + sl] = banks[b][2]
            m = core_of == c
            e_row = row_all[m] - np.array([bk[0] for bk in banks])[bank_of[m]]
            e_b = bank_of[m]
            e_ld = ld_all[m]
            e_w = e_ld // P
            e_dl = e_ld % P
            e_iv = inv_deg[dst[m]]
            order = np.lexsort((e_row, e_b, e_w))
            key = (e_w * nbank + e_b)[order]
            starts = np.r_[0, np.flatnonzero(np.diff(key)) + 1]
            sizes = np.diff(np.r_[starts, key.size])
            rank = np.arange(key.size) - np.repeat(starts, sizes)
            pos = off_wb[e_w[order], e_b[order]] + rank
            stream[pos] = e_row[order].astype(np.int16)
            # seg[slot, dst] one-hot with inv_deg folded in, host-built so
            # the device never runs 16-bit DVE ops next to SWDGE (they lock
            # the descriptor rings and serialize the gathers)
            segm[pos // P, pos % P, e_dl[order]] = e_iv[order]
            percore[c][f"idx_{tag}"] = _pack_idx(stream)
            percore[c][f"seg_{tag}"] = np.ascontiguousarray(
                segm.transpose(1, 0, 2)).astype(BF16)

        S["rels"].append(dict(r=r, snt=snt, dnt=dnt, tag=tag, NW=NW,
                              banks=banks, gathers=gathers, nslots=nslots,
                              nblk_tot=nblk_tot, maxblk=maxblk))

    for nt in NTYPES:
        x = np.asarray(inputs["x_" + nt])
        for c in range(ncore):
            sh = shard[nt]
            percore[c][f"xT_{nt}"] = np.ascontiguousarray(
                x[c * sh:(c + 1) * sh].T).astype(BF16)

    com = dict()
    for nt in NTYPES:
        com[f"We_{nt}"] = np.asarray(inputs["We_" + nt]).astype(BF16)
        com[f"be_{nt}"] = np.asarray(inputs["be_" + nt]).astype(
            np.float32).reshape(-1, 1)
    for l in range(3):
        com[f"W{l}"] = np.asarray(inputs[f"W{l}"]).astype(BF16)
        com[f"L{l}"] = np.asarray(inputs[f"L{l}"]).astype(BF16)
        com[f"b{l}"] = np.asarray(inputs[f"b{l}"]).astype(np.float32).reshape(-1, 1)
    for c in range(ncore):
        percore[c].update(com)
    return S, percore


# ---------------------------------------------------------------------------
# device program
# ---------------------------------------------------------------------------

def build(S):
    cfg = S["cfg"]
    ncore = cfg["NCORE"]
    nw, shard = S["nw"], S["shard"]
    DH, DOUT = cfg["D_H"], cfg["D_OUT"]
    NREL = len(cfg["RELS"])
    nsh_tot = sum(shard.values())
    rel_by_tag = {R["tag"]: R for R in S["rels"]}
    maxblk_all = max(R["maxblk"] for R in S["rels"])
    # praw16 column offsets per phase (buffers reused across phases)
    praw_off = {}
    praw_cols = 0
    for dnt, tags in PHASES:
        o = 0
        for tg in tags:
            praw_off[tg] = o
            o += nw[dnt] * P
        praw_cols = max(praw_cols, o)

    nc = bacc.Bacc("TRN2", target_bir_lowering=False, debug=False,
                   num_devices=ncore, num_swdge_queues=NQ)

    par = {}
    for nt in NTYPES:
        par[f"xT_{nt}"] = nc.declare_dram_parameter(
            f"xT_{nt}", [cfg["MOD"][nt], shard[nt]], mybir.dt.bfloat16, False)
        par[f"We_{nt}"] = nc.declare_dram_parameter(
            f"We_{nt}", [cfg["MOD"][nt], cfg["D_IN"]], mybir.dt.bfloat16, False)
        par[f"be_{nt}"] = nc.declare_dram_parameter(
            f"be_{nt}", [cfg["D_IN"], 1], mybir.dt.float32, False)
    for l in range(3):
        od = DOUT if l == 2 else DH
        par[f"W{l}"] = nc.declare_dram_parameter(
            f"W{l}", [NREL, DH, od], mybir.dt.bfloat16, False)
        par[f"L{l}"] = nc.declare_dram_parameter(
            f"L{l}", [DH, od], mybir.dt.bfloat16, False)
        par[f"b{l}"] = nc.declare_dram_parameter(
            f"b{l}", [od, 1], mybir.dt.float32, False)
    for R in S["rels"]:
        tg = R["tag"]
        par[f"idx_{tg}"] = nc.declare_dram_parameter(
            f"idx_{tg}", [P, R["nslots"] // 16], mybir.dt.int16, False)
        par[f"seg_{tg}"] = nc.declare_dram_parameter(
            f"seg_{tg}", [P, R["nblk_tot"], P], mybir.dt.bfloat16, False)
    out_par = nc.declare_dram_parameter("out", [nsh_tot, DOUT],
                                        mybir.dt.float32, True)

    agin, tabs = {}, {}
    for l in range(3):
        for nt in SRC_NTYPES:
            agin[(l, nt)] = nc.dram_tensor(
                f"agin{l}_{nt}", [shard[nt], DH], mybir.dt.bfloat16)
            tabs[(l, nt)] = nc.dram_tensor(
                f"tab{l}_{nt}", [cfg["N"][nt] + 2, DH], mybir.dt.bfloat16,
                addr_space="Shared")

    with ExitStack() as ctx:
        tc = ctx.enter_context(tile.TileContext(nc))
        nc.gpsimd.load_library(library_config.mlp)

        const = ctx.enter_context(tc.tile_pool(name="const", bufs=1))
        persist = ctx.enter_context(tc.tile_pool(name="persist", bufs=1))
        gpool = ctx.enter_context(tc.tile_pool(name="gpool", bufs=8))
        ipool = ctx.enter_context(tc.tile_pool(name="ipool", bufs=10))
        xpool = ctx.enter_context(tc.tile_pool(name="xpool", bufs=2))
        wpool = ctx.enter_context(tc.tile_pool(name="wpool", bufs=4))
        spool = ctx.enter_context(tc.tile_pool(name="spool", bufs=8))
        pst = ctx.enter_context(tc.tile_pool(name="pst", bufs=2, space="PSUM"))
        psA = ctx.enter_context(tc.tile_pool(name="psA", bufs=2, space="PSUM"))
        psB = ctx.enter_context(tc.tile_pool(name="psB", bufs=2, space="PSUM"))
        psE = ctx.enter_context(tc.tile_pool(name="psE", bufs=2, space="PSUM"))

        identity = const.tile([P, P], mybir.dt.float32)
        from concourse.masks import make_identity
        make_identity(nc, identity[:])
        identity16 = const.tile([P, P], mybir.dt.bfloat16)
        nc.vector.tensor_copy(identity16[:], identity[:])

        sb_W, sb_L, sb_b = {}, {}, {}
        for l in range(3):
            od = DOUT if l == 2 else DH
            t = const.tile([DH, NREL, od], mybir.dt.bfloat16, tag=f"W{l}")
            nc.sync.dma_start(t[:], par[f"W{l}"][:].rearrange("r k o -> k r o"))
            sb_W[l] = t
            sb_L[l] = const.tile([DH, od], mybir.dt.bfloat16, tag=f"L{l}",
                                 name=f"L{l}")
            nc.sync.dma_start(sb_L[l][:], par[f"L{l}"][:])
            sb_b[l] = const.tile([od, 1], mybir.dt.float32, tag=f"b{l}",
                                 name=f"b{l}")
            nc.sync.dma_start(sb_b[l][:], par[f"b{l}"][:])

        zrow = const.tile([1, DH], mybir.dt.bfloat16)
        nc.vector.memset(zrow[:], 0.0)
        for l in range(3):
            for nt in SRC_NTYPES:
                n = cfg["N"][nt]
                nc.sync.dma_start(tabs[(l, nt)][0:1, :], zrow[:])
                nc.sync.dma_start(tabs[(l, nt)][n + 1:n + 2, :], zrow[:])

        hT = [persist.tile([DH, nsh_tot], mybir.dt.bfloat16, tag=f"hT{i}",
                           name=f"hT{i}")
              for i in range(2)]
        nt_off, o = {}, 0
        for nt in NTYPES:
            nt_off[nt] = o
            o += shard[nt]
        praw = persist.tile([DH, praw_cols], mybir.dt.bfloat16, tag="praw")

        gq_counter = [0]

        def emit_embedding(nt):
            mod, sh = cfg["MOD"][nt], shard[nt]
            kt = mod // P
            sb_we = xpool.tile([P, 8, cfg["D_IN"]], mybir.dt.bfloat16, tag="we")
            nc.sync.dma_start(
                sb_we[:, :kt, :],
                par[f"We_{nt}"][:].rearrange("(k p) f -> p k f", p=P))
            sb_be = wpool.tile([cfg["D_IN"], 1], mybir.dt.float32, tag="be")
            nc.sync.dma_start(sb_be[:], par[f"be_{nt}"][:])
            for n0 in range(0, sh, 512):
                n1 = min(n0 + 512, sh)
                cols = n1 - n0
                xt = xpool.tile([P, 8, 512], mybir.dt.bfloat16, tag="xt")
                nc.sync.dma_start(
                    xt[:, :kt, :cols],
                    par[f"xT_{nt}"][:].rearrange(
                        "(k p) n -> p k n", p=P)[:, :, n0:n1])
                pe = psE.tile([P, 512], mybir.dt.float32, tag="emb")
                for k in range(kt):
                    nc.tensor.matmul(pe[:, :cols], sb_we[:, k, :],
                                     xt[:, k, :cols],
                                     start=(k == 0), stop=(k == kt - 1))
                nc.scalar.activation(
                    hT[0][:, nt_off[nt] + n0:nt_off[nt] + n1], pe[:, :cols],
                    mybir.ActivationFunctionType.Identity, bias=sb_be[:])

        def stage_ag_window(l, nt, w0, cols):
            """Transpose one hT[l] window of ntype nt into the AllGather
            staging buffer."""
            src = hT[l % 2][:, nt_off[nt] + w0:nt_off[nt] + w0 + cols]
            pt = pst.tile([P, P], mybir.dt.bfloat16, tag="tp", name="pt16")
            nc.tensor.transpose(pt[:cols, :DH], src, identity16[:])
            stg = wpool.tile([P, DH], mybir.dt.bfloat16, tag="agstg")
            nc.vector.tensor_copy(stg[:cols, :], pt[:cols, :DH])
            nc.sync.dma_start(agin[(l, nt)][w0:w0 + cols, :], stg[:cols, :])

        def emit_ag_collective(l, nt):
            nc.gpsimd.collective_compute(
                "AllGather", mybir.AluOpType.bypass,
                replica_groups=[list(range(ncore))],
                ins=[agin[(l, nt)][:]],
                outs=[tabs[(l, nt)][1:cfg["N"][nt] + 1]],
            )

        def emit_ag(l, nt):
            for w0 in range(0, shard[nt], P):
                stage_ag_window(l, nt, w0, min(P, shard[nt] - w0))
            emit_ag_collective(l, nt)

        def emit_window(l, dnt, tags, w):
            """One dst window: per-relation gathers + seg matmuls into PSUM
            agg -> praw, then W_r matmuls + self-loop + activation."""
            od = DOUT if l == 2 else DH
            sh = shard[dnt]
            cs = nt_off[dnt] + w * P
            ce = min(cs + P, nt_off[dnt] + sh)
            cols = ce - cs
            live = []
            for tg in tags:
                R = rel_by_tag[tg]
                gs = [g for g in R["gathers"] if g[0] == w]
                nmm = sum(g[4] for g in gs)
                if nmm == 0:
                    nc.vector.memset(
                        praw[:, praw_off[tg] + w * P:
                         praw_off[tg] + (w + 1) * P], 0.0)
                    continue
                pa = psA.tile([P, P], mybir.dt.float32, tag="agg")
                mm = 0
                for (_, b, soff, slots, nb, segoff) in gs:
                    sbi = ipool.tile([P, maxblk_all * P // 16],
                                     mybir.dt.int16, tag="idx")
                    nc.sync.dma_start(
                        sbi[:, :slots // 16],
                        par[f"idx_{tg}"][:, soff // 16:(soff + slots) // 16])
                    gt = gpool.tile([P, maxblk_all, P], mybir.dt.bfloat16,
                                    tag="gat")
                    b0, b1, _ = R["banks"][b]
                    q = gq_counter[0] % NQ
                    gq_counter[0] += 1
                    nc.gpsimd.dma_gather(
                        out_ap=gt[:, :nb, :], in_ap=tabs[(l, R["snt"])][b0:b1],
                        idxs_ap=sbi[:, :slots // 16],
                        num_idxs=slots, num_idxs_reg=slots,
                        elem_size=DH, transpose=False, single_packet=False,
                        queue_num=q)
                    sg = spool.tile([P, maxblk_all, P], mybir.dt.bfloat16,
                                    tag="sg")
                    nc.sync.dma_start(
                        sg[:, :nb, :],
                        par[f"seg_{tg}"][:, segoff:segoff + nb, :])
                    for k in range(nb):
                        nc.tensor.matmul(pa[:, :], gt[:, k, :], sg[:, k, :],
                                         start=(mm == 0), stop=(mm == nmm - 1))
                        mm += 1
                live.append((tg, pa))
            for tg, pa in live:
                with nc.allow_low_precision(reason="praw is consumed by a "
                                            "bf16 matmul"):
                    nc.vector.tensor_copy(
                        praw[:, praw_off[tg] + w * P:
                             praw_off[tg] + (w + 1) * P], pa[:, :])
            pb = psB.tile([P, P], mybir.dt.float32, tag="out2")
            for ti, tg in enumerate(tags):
                R = rel_by_tag[tg]
                nc.tensor.matmul(
                    pb[:od, :cols], sb_W[l][:, R["r"], :],
                    praw[:, praw_off[tg] + w * P:praw_off[tg] + w * P + cols],
                    start=(ti == 0), stop=False)
            nc.tensor.matmul(pb[:od, :cols], sb_L[l][:], hT[l % 2][:, cs:ce],
                             start=False, stop=True)
            if l < 2:
                nc.scalar.activation(
                    hT[(l + 1) % 2][:od, cs:ce], pb[:od, :cols],
                    mybir.ActivationFunctionType.Relu, bias=sb_b[l][:])
                if dnt in SRC_NTYPES:
                    stage_ag_window(l + 1, dnt, w * P, cols)
            else:
                fin = wpool.tile([P, P], mybir.dt.float32, tag="fin")
                nc.scalar.activation(
                    fin[:od, :cols], pb[:od, :cols],
                    mybir.ActivationFunctionType.Identity, bias=sb_b[l][:])
                pt = pst.tile([P, P], mybir.dt.float32, tag="tp")
                nc.tensor.transpose(pt[:cols, :od], fin[:od, :cols],
                                    identity[:od, :od])
                stg = wpool.tile([P, DOUT], mybir.dt.float32, tag="ostg")
                nc.vector.tensor_copy(stg[:cols, :], pt[:cols, :od])
                nc.sync.dma_start(out_par[cs:ce, :], stg[:cols, :])

        def emit_phase(l, dnt, tags):
            for w in range(nw[dnt]):
                emit_window(l, dnt, tags, w)

        # ---- program ----
        # Each window's activation stages its next-layer AllGather input;
        # the collective for an ntype is dispatched right after that ntype's
        # windows, so its wire time overlaps the remaining phases' gathers.
        emit_embedding("drug")
        emit_ag(0, "drug")
        emit_embedding("gene")
        emit_ag(0, "gene")
        emit_embedding("disease")
        for l in range(3):
            emit_phase(l, *PHASES[0])
            if l < 2:
                emit_ag_collective(l + 1, "drug")
            emit_phase(l, *PHASES[1])
            if l < 2:
                emit_ag_collective(l + 1, "gene")
            emit_phase(l, *PHASES[2])

    nc.compile()
    return nc


# ---------------------------------------------------------------------------
# entry point
# ---------------------------------------------------------------------------

def _install_ntff_hook():
    if "antenv.axon_hooks" in sys.modules:
        return
    mod = types.ModuleType("antenv.axon_hooks")
    mod._hook = None
    mod.set_axon_ntff_profile_hook = lambda h: setattr(mod, "_hook", h)
    mod.get_axon_ntff_profile_hook = lambda: mod._hook
    sys.modules["antenv.axon_hooks"] = mod
    try:
        import antenv
        antenv.axon_hooks = mod
        from trn_agent_boot.trn_boot import _ntff_profile_via_ctypes
        hook = _ntff_profile_via_ctypes("/opt/axon/libaxon_pjrt.so")
        if hook is not None:
            mod.set_axon_ntff_profile_hook(hook)
    except Exception:
        pass


def run(inputs, cfg=CFG, trace=False, tmpdir=None):
    S, percore = preprocess(cfg, inputs)
    nc = build(S)
    _install_ntff_hook()
    from concourse import bass_utils
    bass_utils.upload_artifacts = lambda d: d
    res = bass_utils.run_bass_kernel_spmd(
        nc, percore, list(range(cfg["NCORE"])), trace=trace, tmpdir=tmpdir,
        trace_cores=[0] if trace else None)
    ncore = cfg["NCORE"]
    shard = {nt: cfg["N"][nt] // ncore for nt in NTYPES}
    outs = []
    o = 0
    for nt in NTYPES:
        parts = [res.results[c]["out"][o:o + shard[nt]] for c in range(ncore)]
        outs.append(np.concatenate(parts, 0))
        o += shard[nt]
    full = np.concatenate(outs, 0).astype(np.float32)
    run.last_exec_time_ns = res.exec_time_ns
    return full


def kernel(**inputs):
    return run(inputs)



# revision 15
# speedup vs baseline: 1.2197x; 1.2197x over previous
"""Trainium2 Bass kernel for nn_BaseRGCNHetero (3-layer heterogeneous RGCN).

Strategy (8 NeuronCores, SPMD):
  - Destination-shard the nodes: core c owns rows [c*N/8, (c+1)*N/8) of every
    node type; all edges whose dst is in the shard are processed there, so
    per-relation aggregates need no cross-core reduction.
  - Aggregate-first algebra: agg[dst] = (sum_{e->dst} h[src]) @ W_r * inv_deg,
    sharing one bf16 DRAM gather table per source ntype (drug, gene) per
    layer.  After each layer the drug/gene h-shards are AllGathered (bf16)
    into the next layer's tables; each AllGather is issued as soon as its
    ntype's windows finish so the wire time overlaps the remaining gathers.
  - Edge slot stream per relation: 128-dst windows in natural order, one
    chunk per (window, src bank), slots sorted by src row and padded to a
    multiple of 128 (pad slots point at an all-zero table row).  Non-
    transposed dma_gather pulls h[src] rows node-major ([slot, feat] blocks);
    gathers round-robin over 4 SWDGE queues so descriptor generation runs on
    all four GpSimd Q7 core pairs concurrently (3.3x single-queue, and the
    non-transpose path avoids the xbar that makes concurrent transposed
    gathers corrupt each other).
  - Segment sums on TensorE: per 128-slot block, VectorE builds a one-hot
    seg matrix seg[slot, dst] = (dstcol[slot] == dst) * inv_deg[slot] from
    host-streamed per-block columns; matmul(gt_block^T @ seg) accumulates
    agg[feat, dst] for the window in PSUM across the window's blocks.
  - Per dst window: agg -> bf16 praw, then one matmul per relation (W_r) plus
    the self-loop h @ L accumulate in a single PSUM bank; bias (+relu) is a
    fused ScalarE activation per window.
"""
import os
import sys
import types
import numpy as np
import ml_dtypes
from contextlib import ExitStack



import concourse.bass as bass
import concourse.bacc as bacc
import concourse.tile as tile
from concourse import mybir, library_config

BF16 = ml_dtypes.bfloat16
FP8 = ml_dtypes.float8_e4m3
P = 128
NQ = 4             # SWDGE queues (gather descriptor-gen parallelism)
CHUNK = 4          # gather call granularity in 128-slot blocks (512 descs =
                   # half a ring, so the ring holds two chunks and desc-gen
                   # pipelines ahead of the drain; chunks round-robin queues
                   # so all 4 rings drain concurrently)

CFG = dict(
    N={"drug": 20000, "gene": 50000, "disease": 10000},
    MOD={"drug": 1024, "gene": 768, "disease": 512},
    D_IN=128, D_H=128, D_OUT=64,
    RELS=[("drug", "disease", "dd"), ("drug", "drug", "ddr"),
          ("drug", "gene", "dg"), ("gene", "disease", "gd"),
          ("gene", "gene", "gg")],
    NCORE=8,
    BANK=32768,     # dma_gather int16 row-index limit per table slice
)

NTYPES = ("drug", "gene", "disease")
SRC_NTYPES = ("drug", "gene")
# layer processing phases: dst ntype -> relations feeding it (tags).
# drug first: its AllGather (issued right after the drug windows) then hides
# under the gene+disease phases; the gene AllGather hides under disease.
PHASES = [("drug", ["ddr"]), ("gene", ["dg", "gg"]), ("disease", ["dd", "gd"])]


# ---------------------------------------------------------------------------
# host-side preprocessing
# ---------------------------------------------------------------------------

def _pack_idx(stream):
    """int array (len % 128 == 0) -> dma_gather idx layout [128, len/16] int16:
    idx i at (i%16, i//16), replicated across the 8 groups of 16 partitions."""
    n = stream.size
    v = stream.astype(np.int16).reshape(n // 16, 16).T
    return np.tile(v, (8, 1))


def _banks(cfg, snt):
    """Gather-table bank slices for source ntype snt.
    Table rows: 0 = zeros, 1..N = nodes, N+1 = zeros.
    Returns list of (start_row, end_row, pad_row_relative)."""
    n = cfg["N"][snt]
    trows = n + 2
    if trows <= cfg["BANK"]:
        return [(0, trows, 0)]
    return [(0, cfg["BANK"], 0), (cfg["BANK"], trows, n + 1 - cfg["BANK"])]


def preprocess(cfg, inputs):
    ncore = cfg["NCORE"]
    shard = {nt: cfg["N"][nt] // ncore for nt in NTYPES}
    nw = {nt: -(-shard[nt] // P) for nt in NTYPES}

    S = dict(cfg=cfg, nw=nw, shard=shard, rels=[])
    percore = [dict() for _ in range(ncore)]

    for r, (snt, dnt, tag) in enumerate(cfg["RELS"]):
        src = np.asarray(inputs["e_" + tag + "_s"]).astype(np.int64)
        dst = np.asarray(inputs["e_" + tag + "_d"]).astype(np.int64)
        banks = _banks(cfg, snt)
        nbank = len(banks)
        NW = nw[dnt]
        dsh = shard[dnt]

        core_of = dst // dsh
        deg_all = np.bincount(dst, minlength=cfg["N"][dnt]).astype(np.float32)
        inv_deg = 1.0 / np.maximum(deg_all, 1.0)

        row_all = src + 1
        bank_of = (row_all >= cfg["BANK"]).astype(np.int64) if nbank == 2 \
            else np.zeros(row_all.size, np.int64)

        # per-core per-(window, bank) edge counts -> shared block counts
        cnt = np.zeros((ncore, NW, nbank), np.int64)
        ld_all = dst - core_of * dsh
        w_all = ld_all // P
        for c in range(ncore):
            m = core_of == c
            key = w_all[m] * nbank + bank_of[m]
            cnt[c] = np.bincount(key, minlength=NW * nbank).reshape(NW, nbank)
        nblk = -(-cnt.max(axis=0) // P)          # [NW, nbank] shared
        slots_wb = nblk * P
        off_wb = np.zeros((NW, nbank), np.int64)
        gathers = []                              # (bank, off, slots, nblk, segoff)
        off = 0
        segoff = 0
        for w in range(NW):
            for b in range(nbank):
                if nblk[w, b] == 0:
                    continue
                off_wb[w, b] = off
                gathers.append((w, b, int(off), int(slots_wb[w, b]),
                                int(nblk[w, b]), int(segoff)))
                off += int(slots_wb[w, b])
                segoff += int(nblk[w, b])
        nslots = max(off, P)
        nblk_tot = max(segoff, 1)
        maxblk = int(nblk.max()) if nblk.size else 1

        for c in range(ncore):
            stream = np.zeros(nslots, np.int16)
            segm = np.zeros((nblk_tot, P, P), np.float32)
            for w, b, o, sl, nb, so in gathers:
                stream[o:o + sl] = banks[b][2]
            m = core_of == c
            e_row = row_all[m] - np.array([bk[0] for bk in banks])[bank_of[m]]
            e_b = bank_of[m]
            e_ld = ld_all[m]
            e_w = e_ld // P
            e_dl = e_ld % P
            order = np.lexsort((e_row, e_b, e_w))
            key = (e_w * nbank + e_b)[order]
            starts = np.r_[0, np.flatnonzero(np.diff(key)) + 1]
            sizes = np.diff(np.r_[starts, key.size])
            rank = np.arange(key.size) - np.repeat(starts, sizes)
            pos = off_wb[e_w[order], e_b[order]] + rank
            stream[pos] = e_row[order].astype(np.int16)
            # seg[slot, dst] pure one-hot (exact in fp8); inv_deg is applied
            # on-device as a per-dst-partition scale after the swapped
            # (dst-major) segment matmul.
            segm[pos // P, pos % P, e_dl[order]] = 1.0
            percore[c][f"idx_{tag}"] = _pack_idx(stream)
            percore[c][f"seg_{tag}"] = np.ascontiguousarray(
                segm.transpose(1, 0, 2)).astype(FP8)
            # per-window inv_deg columns for this relation: invd[dl, w]
            dsh0 = c * dsh
            iv = np.ones((NW * P,), np.float32)
            nloc = min(dsh, cfg["N"][dnt] - dsh0)
            iv[:nloc] = inv_deg[dsh0:dsh0 + nloc]
            percore[c][f"invd_{tag}"] = np.ascontiguousarray(
                iv.reshape(NW, P).T)

        S["rels"].append(dict(r=r, snt=snt, dnt=dnt, tag=tag, NW=NW,
                              banks=banks, gathers=gathers, nslots=nslots,
                              nblk_tot=nblk_tot, maxblk=maxblk))

    for nt in NTYPES:
        x = np.asarray(inputs["x_" + nt])
        for c in range(ncore):
            sh = shard[nt]
            percore[c][f"xT_{nt}"] = np.ascontiguousarray(
                x[c * sh:(c + 1) * sh].T).astype(BF16)

    com = dict()
    for nt in NTYPES:
        com[f"We_{nt}"] = np.asarray(inputs["We_" + nt]).astype(BF16)
        com[f"be_{nt}"] = np.asarray(inputs["be_" + nt]).astype(
            np.float32).reshape(-1, 1)
    for l in range(3):
        com[f"W{l}"] = np.asarray(inputs[f"W{l}"]).astype(BF16)
        com[f"L{l}"] = np.asarray(inputs[f"L{l}"]).astype(BF16)
        com[f"b{l}"] = np.asarray(inputs[f"b{l}"]).astype(np.float32).reshape(-1, 1)
    for c in range(ncore):
        percore[c].update(com)
    return S, percore


# ---------------------------------------------------------------------------
# device program
# ---------------------------------------------------------------------------

def build(S):
    cfg = S["cfg"]
    ncore = cfg["NCORE"]
    nw, shard = S["nw"], S["shard"]
    DH, DOUT = cfg["D_H"], cfg["D_OUT"]
    NREL = len(cfg["RELS"])
    nsh_tot = sum(shard.values())
    rel_by_tag = {R["tag"]: R for R in S["rels"]}
    maxblk_all = max(R["maxblk"] for R in S["rels"])
    # praw16 column offsets per phase (buffers reused across phases)
    praw_off = {}
    praw_cols = 0
    for dnt, tags in PHASES:
        o = 0
        for tg in tags:
            praw_off[tg] = o
            o += nw[dnt] * P
        praw_cols = max(praw_cols, o)

    nc = bacc.Bacc("TRN2", target_bir_lowering=False, debug=False,
                   num_devices=ncore, num_swdge_queues=NQ,
                   dynamic_dma_scratch_size=32768)

    par = {}
    for nt in NTYPES:
        par[f"xT_{nt}"] = nc.declare_dram_parameter(
            f"xT_{nt}", [cfg["MOD"][nt], shard[nt]], mybir.dt.bfloat16, False)
        par[f"We_{nt}"] = nc.declare_dram_parameter(
            f"We_{nt}", [cfg["MOD"][nt], cfg["D_IN"]], mybir.dt.bfloat16, False)
        par[f"be_{nt}"] = nc.declare_dram_parameter(
            f"be_{nt}", [cfg["D_IN"], 1], mybir.dt.float32, False)
    for l in range(3):
        od = DOUT if l == 2 else DH
        par[f"W{l}"] = nc.declare_dram_parameter(
            f"W{l}", [NREL, DH, od], mybir.dt.bfloat16, False)
        par[f"L{l}"] = nc.declare_dram_parameter(
            f"L{l}", [DH, od], mybir.dt.bfloat16, False)
        par[f"b{l}"] = nc.declare_dram_parameter(
            f"b{l}", [od, 1], mybir.dt.float32, False)
    for R in S["rels"]:
        tg = R["tag"]
        par[f"idx_{tg}"] = nc.declare_dram_parameter(
            f"idx_{tg}", [P, R["nslots"] // 16], mybir.dt.int16, False)
        par[f"seg_{tg}"] = nc.declare_dram_parameter(
            f"seg_{tg}", [P, R["nblk_tot"], P], mybir.dt.float8e4, False)
        par[f"invd_{tg}"] = nc.declare_dram_parameter(
            f"invd_{tg}", [P, R["NW"]], mybir.dt.float32, False)
    out_par = nc.declare_dram_parameter("out", [nsh_tot, DOUT],
                                        mybir.dt.float32, True)

    agin, tabs = {}, {}
    for l in range(3):
        for nt in SRC_NTYPES:
            agin[(l, nt)] = nc.dram_tensor(
                f"agin{l}_{nt}", [shard[nt], DH], mybir.dt.bfloat16)
            tabs[(l, nt)] = nc.dram_tensor(
                f"tab{l}_{nt}", [cfg["N"][nt] + 2, DH], mybir.dt.bfloat16,
                addr_space="Shared")

    with ExitStack() as ctx:
        tc = ctx.enter_context(tile.TileContext(nc))
        nc.gpsimd.load_library(library_config.mlp)

        const = ctx.enter_context(tc.tile_pool(name="const", bufs=1))
        persist = ctx.enter_context(tc.tile_pool(name="persist", bufs=1))
        gpool = ctx.enter_context(tc.tile_pool(name="gpool", bufs=12))
        ipool = ctx.enter_context(tc.tile_pool(name="ipool", bufs=10))
        xpool = ctx.enter_context(tc.tile_pool(name="xpool", bufs=2))
        wpool = ctx.enter_context(tc.tile_pool(name="wpool", bufs=4))
        spool = ctx.enter_context(tc.tile_pool(name="spool", bufs=12))
        pst = ctx.enter_context(tc.tile_pool(name="pst", bufs=2, space="PSUM"))
        psA = ctx.enter_context(tc.tile_pool(name="psA", bufs=2, space="PSUM"))
        psB = ctx.enter_context(tc.tile_pool(name="psB", bufs=2, space="PSUM"))
        psE = ctx.enter_context(tc.tile_pool(name="psE", bufs=2, space="PSUM"))

        identity = const.tile([P, P], mybir.dt.float32)
        from concourse.masks import make_identity
        make_identity(nc, identity[:])
        identity16 = const.tile([P, P], mybir.dt.bfloat16)
        nc.vector.tensor_copy(identity16[:], identity[:])

        sb_W, sb_L, sb_b = {}, {}, {}
        for l in range(3):
            od = DOUT if l == 2 else DH
            t = const.tile([DH, NREL, od], mybir.dt.bfloat16, tag=f"W{l}")
            nc.sync.dma_start(t[:], par[f"W{l}"][:].rearrange("r k o -> k r o"))
            sb_W[l] = t
            sb_L[l] = const.tile([DH, od], mybir.dt.bfloat16, tag=f"L{l}",
                                 name=f"L{l}")
            nc.sync.dma_start(sb_L[l][:], par[f"L{l}"][:])
            sb_b[l] = const.tile([od, 1], mybir.dt.float32, tag=f"b{l}",
                                 name=f"b{l}")
            nc.sync.dma_start(sb_b[l][:], par[f"b{l}"][:])

        sb_invd = {}
        for R in S["rels"]:
            tg = R["tag"]
            t = const.tile([P, R["NW"]], mybir.dt.float32, tag=f"invd_{tg}",
                           name=f"invd_{tg}")
            nc.sync.dma_start(t[:], par[f"invd_{tg}"][:])
            sb_invd[tg] = t

        zrow = const.tile([1, DH], mybir.dt.bfloat16)
        nc.vector.memset(zrow[:], 0.0)
        for l in range(3):
            for nt in SRC_NTYPES:
                n = cfg["N"][nt]
                nc.sync.dma_start(tabs[(l, nt)][0:1, :], zrow[:])
                nc.sync.dma_start(tabs[(l, nt)][n + 1:n + 2, :], zrow[:])

        # zero the gather buffers once: trailing-pad descriptors are trimmed
        # (idx -1), so untouched slots must hold finite bf16 (0 x seg-zero).
        for _ in range(12):
            g0 = gpool.tile([P, CHUNK, P], mybir.dt.bfloat16, tag="gat")
            nc.vector.memset(g0[:], 0.0)

        hT = [persist.tile([DH, nsh_tot], mybir.dt.bfloat16, tag=f"hT{i}",
                           name=f"hT{i}")
              for i in range(2)]
        nt_off, o = {}, 0
        for nt in NTYPES:
            nt_off[nt] = o
            o += shard[nt]
        praw = persist.tile([DH, praw_cols], mybir.dt.bfloat16, tag="praw")

        gq_counter = [0]

        def emit_embedding(nt):
            mod, sh = cfg["MOD"][nt], shard[nt]
            kt = mod // P
            sb_we = xpool.tile([P, 8, cfg["D_IN"]], mybir.dt.bfloat16, tag="we")
            nc.sync.dma_start(
                sb_we[:, :kt, :],
                par[f"We_{nt}"][:].rearrange("(k p) f -> p k f", p=P))
            sb_be = wpool.tile([cfg["D_IN"], 1], mybir.dt.float32, tag="be")
            nc.sync.dma_start(sb_be[:], par[f"be_{nt}"][:])
            for n0 in range(0, sh, 512):
                n1 = min(n0 + 512, sh)
                cols = n1 - n0
                xt = xpool.tile([P, 8, 512], mybir.dt.bfloat16, tag="xt")
                nc.sync.dma_start(
                    xt[:, :kt, :cols],
                    par[f"xT_{nt}"][:].rearrange(
                        "(k p) n -> p k n", p=P)[:, :, n0:n1])
                pe = psE.tile([P, 512], mybir.dt.float32, tag="emb")
                for k in range(kt):
                    nc.tensor.matmul(pe[:, :cols], sb_we[:, k, :],
                                     xt[:, k, :cols],
                                     start=(k == 0), stop=(k == kt - 1))
                nc.scalar.activation(
                    hT[0][:, nt_off[nt] + n0:nt_off[nt] + n1], pe[:, :cols],
                    mybir.ActivationFunctionType.Identity, bias=sb_be[:])

        def stage_ag_window(l, nt, w0, cols):
            """Transpose one hT[l] window of ntype nt into the AllGather
            staging buffer."""
            src = hT[l % 2][:, nt_off[nt] + w0:nt_off[nt] + w0 + cols]
            pt = pst.tile([P, P], mybir.dt.bfloat16, tag="tp", name="pt16")
            nc.tensor.transpose(pt[:cols, :DH], src, identity16[:])
            stg = wpool.tile([P, DH], mybir.dt.bfloat16, tag="agstg")
            nc.vector.tensor_copy(stg[:cols, :], pt[:cols, :DH])
            nc.sync.dma_start(agin[(l, nt)][w0:w0 + cols, :], stg[:cols, :])

        def emit_ag_collective(l, nt):
            nc.gpsimd.collective_compute(
                "AllGather", mybir.AluOpType.bypass,
                replica_groups=[list(range(ncore))],
                ins=[agin[(l, nt)][:]],
                outs=[tabs[(l, nt)][1:cfg["N"][nt] + 1]],
            )

        def emit_ag(l, nt):
            for w0 in range(0, shard[nt], P):
                stage_ag_window(l, nt, w0, min(P, shard[nt] - w0))
            emit_ag_collective(l, nt)

        def emit_window(l, dnt, tags, w):
            """One dst window: per-relation gathers + seg matmuls into PSUM
            agg -> praw, then W_r matmuls + self-loop + activation."""
            od = DOUT if l == 2 else DH
            sh = shard[dnt]
            cs = nt_off[dnt] + w * P
            ce = min(cs + P, nt_off[dnt] + sh)
            cols = ce - cs
            live = []
            for tg in tags:
                R = rel_by_tag[tg]
                gs = [g for g in R["gathers"] if g[0] == w]
                nmm = sum(g[4] for g in gs)
                if nmm == 0:
                    nc.vector.memset(
                        praw[:, praw_off[tg] + w * P:
                         praw_off[tg] + (w + 1) * P], 0.0)
                    continue
                pa = psA.tile([P, P], mybir.dt.float32, tag="agg")
                mm = 0
                for (_, b, soff, slots, nb, segoff) in gs:
                    sbi = ipool.tile([P, maxblk_all * P // 16],
                                     mybir.dt.int16, tag="idx")
                    nc.sync.dma_start(
                        sbi[:, :slots // 16],
                        par[f"idx_{tg}"][:, soff // 16:(soff + slots) // 16])
                    b0, b1, _ = R["banks"][b]
                    for k0 in range(0, nb, CHUNK):
                        nbc = min(CHUNK, nb - k0)
                        csl = nbc * P
                        gt = gpool.tile([P, CHUNK, P], mybir.dt.bfloat16,
                                        tag="gat")
                        q = gq_counter[0] % NQ
                        gq_counter[0] += 1
                        nc.gpsimd.dma_gather(
                            out_ap=gt[:, :nbc, :],
                            in_ap=tabs[(l, R["snt"])][b0:b1],
                            idxs_ap=sbi[:, k0 * 8:k0 * 8 + csl // 16],
                            num_idxs=csl, num_idxs_reg=csl,
                            elem_size=DH, transpose=False, single_packet=True,
                            queue_num=q)
                        sg = spool.tile([P, CHUNK, P], mybir.dt.float8e4,
                                        tag="sg")
                        nc.sync.dma_start(
                            sg[:, :nbc, :],
                            par[f"seg_{tg}"][:, segoff + k0:segoff + k0 + nbc, :])
                        for k in range(nbc):
                            # dst-major: pa[dst, feat] = seg^T @ gathered
                            nc.tensor.matmul(pa[:, :], sg[:, k, :], gt[:, k, :],
                                             start=(mm == 0),
                                             stop=(mm == nmm - 1))
                            mm += 1
                live.append((tg, pa))
            for tg, pa in live:
                # inv_deg as per-dst-partition scale while copying to SBUF,
                # then PE-transpose back to the feat-major praw layout
                tr = wpool.tile([P, P], mybir.dt.bfloat16, tag="prawT")
                with nc.allow_low_precision(reason="praw is consumed by a "
                                            "bf16 matmul"):
                    nc.scalar.activation(
                        tr[:cols, :], pa[:cols, :],
                        mybir.ActivationFunctionType.Identity,
                        scale=sb_invd[tg][:cols, w:w + 1])
                pt = pst.tile([P, P], mybir.dt.bfloat16, tag="tp",
                              name="pt16")
                nc.tensor.transpose(pt[:, :cols], tr[:cols, :],
                                    identity16[:cols, :cols])
                nc.vector.tensor_copy(
                    praw[:, praw_off[tg] + w * P:
                         praw_off[tg] + w * P + cols], pt[:, :cols])
            pb = psB.tile([P, P], mybir.dt.float32, tag="out2")
            for ti, tg in enumerate(tags):
                R = rel_by_tag[tg]
                nc.tensor.matmul(
                    pb[:od, :cols], sb_W[l][:, R["r"], :],
                    praw[:, praw_off[tg] + w * P:praw_off[tg] + w * P + cols],
                    start=(ti == 0), stop=False)
            nc.tensor.matmul(pb[:od, :cols], sb_L[l][:], hT[l % 2][:, cs:ce],
                             start=False, stop=True)
            if l < 2:
                nc.scalar.activation(
                    hT[(l + 1) % 2][:od, cs:ce], pb[:od, :cols],
                    mybir.ActivationFunctionType.Relu, bias=sb_b[l][:])
                if dnt in SRC_NTYPES:
                    stage_ag_window(l + 1, dnt, w * P, cols)
            else:
                fin = wpool.tile([P, P], mybir.dt.float32, tag="fin")
                nc.scalar.activation(
                    fin[:od, :cols], pb[:od, :cols],
                    mybir.ActivationFunctionType.Identity, bias=sb_b[l][:])
                pt = pst.tile([P, P], mybir.dt.float32, tag="tp")
                nc.tensor.transpose(pt[:cols, :od], fin[:od, :cols],
                                    identity[:od, :od])
                stg = wpool.tile([P, DOUT], mybir.dt.float32, tag="ostg")
                nc.vector.tensor_copy(stg[:cols, :], pt[:cols, :od])
                nc.sync.dma_start(out_par[cs:ce, :], stg[:cols, :])

        def emit_phase(l, dnt, tags):
            for w in range(nw[dnt]):
                emit_window(l, dnt, tags, w)

        # ---- program ----
        # Each window's activation stages its next-layer AllGather input;
        # the collective for an ntype is dispatched right after that ntype's
        # windows, so its wire time overlaps the remaining phases' gathers.
        emit_embedding("drug")
        emit_ag(0, "drug")
        emit_embedding("gene")
        emit_ag(0, "gene")
        emit_embedding("disease")
        for l in range(3):
            emit_phase(l, *PHASES[0])
            if l < 2:
                emit_ag_collective(l + 1, "drug")
            emit_phase(l, *PHASES[1])
            if l < 2:
                emit_ag_collective(l + 1, "gene")
            emit_phase(l, *PHASES[2])

    nc.compile()
    return nc


# ---------------------------------------------------------------------------
# entry point
# ---------------------------------------------------------------------------

def _install_ntff_hook():
    if "antenv.axon_hooks" in sys.modules:
        return
    mod = types.ModuleType("antenv.axon_hooks")
    mod._hook = None
    mod.set_axon_ntff_profile_hook = lambda h: setattr(mod, "_hook", h)
    mod.get_axon_ntff_profile_hook = lambda: mod._hook
    sys.modules["antenv.axon_hooks"] = mod
    try:
        import antenv
        antenv.axon_hooks = mod
        from trn_agent_boot.trn_boot import _ntff_profile_via_ctypes
        hook = _ntff_profile_via_ctypes("/opt/axon/libaxon_pjrt.so")
        if hook is not None:
            mod.set_axon_ntff_profile_hook(hook)
    except Exception:
        pass


def run(inputs, cfg=CFG, trace=False, tmpdir=None):
    S, percore = preprocess(cfg, inputs)
    nc = build(S)
    _install_ntff_hook()
    from concourse import bass_utils
    bass_utils.upload_artifacts = lambda d: d
    res = bass_utils.run_bass_kernel_spmd(
        nc, percore, list(range(cfg["NCORE"])), trace=trace, tmpdir=tmpdir,
        trace_cores=[0] if trace else None)
    ncore = cfg["NCORE"]
    shard = {nt: cfg["N"][nt] // ncore for nt in NTYPES}
    outs = []
    o = 0
    for nt in NTYPES:
        parts = [res.results[c]["out"][o:o + shard[nt]] for c in range(ncore)]
        outs.append(np.concatenate(parts, 0))
        o += shard[nt]
    full = np.concatenate(outs, 0).astype(np.float32)
    run.last_exec_time_ns = res.exec_time_ns
    return full


def kernel(**inputs):
    return run(inputs)



# revision 16
# speedup vs baseline: 1.2215x; 1.0014x over previous
"""Trainium2 Bass kernel for nn_BaseRGCNHetero (3-layer heterogeneous RGCN).

Strategy (8 NeuronCores, SPMD):
  - Destination-shard the nodes: core c owns rows [c*N/8, (c+1)*N/8) of every
    node type; all edges whose dst is in the shard are processed there, so
    per-relation aggregates need no cross-core reduction.
  - Aggregate-first algebra: agg[dst] = (sum_{e->dst} h[src]) @ W_r * inv_deg,
    sharing one bf16 DRAM gather table per source ntype (drug, gene) per
    layer.  After each layer the drug/gene h-shards are AllGathered (bf16)
    into the next layer's tables; each AllGather is issued as soon as its
    ntype's windows finish so the wire time overlaps the remaining gathers.
  - Edge slot stream per relation: 128-dst windows in natural order, one
    chunk per (window, src bank), slots sorted by src row and padded to a
    multiple of 128 (pad slots point at an all-zero table row).  Non-
    transposed dma_gather pulls h[src] rows node-major ([slot, feat] blocks);
    gathers round-robin over 4 SWDGE queues so descriptor generation runs on
    all four GpSimd Q7 core pairs concurrently (3.3x single-queue, and the
    non-transpose path avoids the xbar that makes concurrent transposed
    gathers corrupt each other).
  - Segment sums on TensorE: per 128-slot block, VectorE builds a one-hot
    seg matrix seg[slot, dst] = (dstcol[slot] == dst) * inv_deg[slot] from
    host-streamed per-block columns; matmul(gt_block^T @ seg) accumulates
    agg[feat, dst] for the window in PSUM across the window's blocks.
  - Per dst window: agg -> bf16 praw, then one matmul per relation (W_r) plus
    the self-loop h @ L accumulate in a single PSUM bank; bias (+relu) is a
    fused ScalarE activation per window.
"""
import os
import sys
import types
import numpy as np
import ml_dtypes
from contextlib import ExitStack

# Fund a larger SWDGE descriptor carveout (ring depth) so gather desc-gen can
# run ahead of the DMA drain; must match Bacc's dynamic_dma_scratch_size.
os.environ.setdefault("TRNINF_DYNAMIC_DMA_SCRATCH_SIZE", "32768")



import concourse.bass as bass
import concourse.bacc as bacc
import concourse.tile as tile
from concourse import mybir, library_config

BF16 = ml_dtypes.bfloat16
FP8 = ml_dtypes.float8_e4m3
P = 128
NQ = 4             # SWDGE queues (gather descriptor-gen parallelism)
CHUNK = 4          # gather call granularity in 128-slot blocks (512 descs =
                   # half a ring, so the ring holds two chunks and desc-gen
                   # pipelines ahead of the drain; chunks round-robin queues
                   # so all 4 rings drain concurrently)

CFG = dict(
    N={"drug": 20000, "gene": 50000, "disease": 10000},
    MOD={"drug": 1024, "gene": 768, "disease": 512},
    D_IN=128, D_H=128, D_OUT=64,
    RELS=[("drug", "disease", "dd"), ("drug", "drug", "ddr"),
          ("drug", "gene", "dg"), ("gene", "disease", "gd"),
          ("gene", "gene", "gg")],
    NCORE=8,
    BANK=32768,     # dma_gather int16 row-index limit per table slice
)

NTYPES = ("drug", "gene", "disease")
SRC_NTYPES = ("drug", "gene")
# layer processing phases: dst ntype -> relations feeding it (tags).
# drug first: its AllGather (issued right after the drug windows) then hides
# under the gene+disease phases; the gene AllGather hides under disease.
PHASES = [("drug", ["ddr"]), ("gene", ["dg", "gg"]), ("disease", ["dd", "gd"])]


# ---------------------------------------------------------------------------
# host-side preprocessing
# ---------------------------------------------------------------------------

def _pack_idx(stream):
    """int array (len % 128 == 0) -> dma_gather idx layout [128, len/16] int16:
    idx i at (i%16, i//16), replicated across the 8 groups of 16 partitions."""
    n = stream.size
    v = stream.astype(np.int16).reshape(n // 16, 16).T
    return np.tile(v, (8, 1))


def _banks(cfg, snt):
    """Gather-table bank slices for source ntype snt.
    Table rows: 0 = zeros, 1..N = nodes, N+1 = zeros.
    Returns list of (start_row, end_row, pad_row_relative)."""
    n = cfg["N"][snt]
    trows = n + 2
    if trows <= cfg["BANK"]:
        return [(0, trows, 0)]
    return [(0, cfg["BANK"], 0), (cfg["BANK"], trows, n + 1 - cfg["BANK"])]


def preprocess(cfg, inputs):
    ncore = cfg["NCORE"]
    shard = {nt: cfg["N"][nt] // ncore for nt in NTYPES}
    nw = {nt: -(-shard[nt] // P) for nt in NTYPES}

    S = dict(cfg=cfg, nw=nw, shard=shard, rels=[])
    percore = [dict() for _ in range(ncore)]

    for r, (snt, dnt, tag) in enumerate(cfg["RELS"]):
        src = np.asarray(inputs["e_" + tag + "_s"]).astype(np.int64)
        dst = np.asarray(inputs["e_" + tag + "_d"]).astype(np.int64)
        banks = _banks(cfg, snt)
        nbank = len(banks)
        NW = nw[dnt]
        dsh = shard[dnt]

        core_of = dst // dsh
        deg_all = np.bincount(dst, minlength=cfg["N"][dnt]).astype(np.float32)
        inv_deg = 1.0 / np.maximum(deg_all, 1.0)

        row_all = src + 1
        bank_of = (row_all >= cfg["BANK"]).astype(np.int64) if nbank == 2 \
            else np.zeros(row_all.size, np.int64)

        # per-core per-(window, bank) edge counts -> shared block counts
        cnt = np.zeros((ncore, NW, nbank), np.int64)
        ld_all = dst - core_of * dsh
        w_all = ld_all // P
        for c in range(ncore):
            m = core_of == c
            key = w_all[m] * nbank + bank_of[m]
            cnt[c] = np.bincount(key, minlength=NW * nbank).reshape(NW, nbank)
        nblk = -(-cnt.max(axis=0) // P)          # [NW, nbank] shared
        slots_wb = nblk * P
        off_wb = np.zeros((NW, nbank), np.int64)
        gathers = []                              # (bank, off, slots, nblk, segoff)
        off = 0
        segoff = 0
        for w in range(NW):
            for b in range(nbank):
                if nblk[w, b] == 0:
                    continue
                off_wb[w, b] = off
                gathers.append((w, b, int(off), int(slots_wb[w, b]),
                                int(nblk[w, b]), int(segoff)))
                off += int(slots_wb[w, b])
                segoff += int(nblk[w, b])
        nslots = max(off, P)
        nblk_tot = max(segoff, 1)
        maxblk = int(nblk.max()) if nblk.size else 1

        for c in range(ncore):
            stream = np.zeros(nslots, np.int16)
            segm = np.zeros((nblk_tot, P, P), np.float32)
            for w, b, o, sl, nb, so in gathers:
                stream[o:o + sl] = banks[b][2]
            m = core_of == c
            e_row = row_all[m] - np.array([bk[0] for bk in banks])[bank_of[m]]
            e_b = bank_of[m]
            e_ld = ld_all[m]
            e_w = e_ld // P
            e_dl = e_ld % P
            order = np.lexsort((e_row, e_b, e_w))
            key = (e_w * nbank + e_b)[order]
            starts = np.r_[0, np.flatnonzero(np.diff(key)) + 1]
            sizes = np.diff(np.r_[starts, key.size])
            rank = np.arange(key.size) - np.repeat(starts, sizes)
            pos = off_wb[e_w[order], e_b[order]] + rank
            stream[pos] = e_row[order].astype(np.int16)
            # seg[slot, dst] pure one-hot (exact in fp8); inv_deg is applied
            # on-device as a per-dst-partition scale after the swapped
            # (dst-major) segment matmul.
            segm[pos // P, pos % P, e_dl[order]] = 1.0
            percore[c][f"idx_{tag}"] = _pack_idx(stream)
            percore[c][f"seg_{tag}"] = np.ascontiguousarray(
                segm.transpose(1, 0, 2)).astype(FP8)
            # per-window inv_deg columns for this relation: invd[dl, w]
            dsh0 = c * dsh
            iv = np.ones((NW * P,), np.float32)
            nloc = min(dsh, cfg["N"][dnt] - dsh0)
            iv[:nloc] = inv_deg[dsh0:dsh0 + nloc]
            percore[c][f"invd_{tag}"] = np.ascontiguousarray(
                iv.reshape(NW, P).T)

        S["rels"].append(dict(r=r, snt=snt, dnt=dnt, tag=tag, NW=NW,
                              banks=banks, gathers=gathers, nslots=nslots,
                              nblk_tot=nblk_tot, maxblk=maxblk))

    for nt in NTYPES:
        x = np.asarray(inputs["x_" + nt])
        for c in range(ncore):
            sh = shard[nt]
            percore[c][f"xT_{nt}"] = np.ascontiguousarray(
                x[c * sh:(c + 1) * sh].T).astype(BF16)

    com = dict()
    for nt in NTYPES:
        com[f"We_{nt}"] = np.asarray(inputs["We_" + nt]).astype(BF16)
        com[f"be_{nt}"] = np.asarray(inputs["be_" + nt]).astype(
            np.float32).reshape(-1, 1)
    for l in range(3):
        com[f"W{l}"] = np.asarray(inputs[f"W{l}"]).astype(BF16)
        com[f"L{l}"] = np.asarray(inputs[f"L{l}"]).astype(BF16)
        com[f"b{l}"] = np.asarray(inputs[f"b{l}"]).astype(np.float32).reshape(-1, 1)
    for c in range(ncore):
        percore[c].update(com)
    return S, percore


# ---------------------------------------------------------------------------
# device program
# ---------------------------------------------------------------------------

def build(S):
    cfg = S["cfg"]
    ncore = cfg["NCORE"]
    nw, shard = S["nw"], S["shard"]
    DH, DOUT = cfg["D_H"], cfg["D_OUT"]
    NREL = len(cfg["RELS"])
    nsh_tot = sum(shard.values())
    rel_by_tag = {R["tag"]: R for R in S["rels"]}
    maxblk_all = max(R["maxblk"] for R in S["rels"])
    # praw16 column offsets per phase (buffers reused across phases)
    praw_off = {}
    praw_cols = 0
    for dnt, tags in PHASES:
        o = 0
        for tg in tags:
            praw_off[tg] = o
            o += nw[dnt] * P
        praw_cols = max(praw_cols, o)

    nc = bacc.Bacc("TRN2", target_bir_lowering=False, debug=False,
                   num_devices=ncore, num_swdge_queues=NQ,
                   dynamic_dma_scratch_size=32768)

    par = {}
    for nt in NTYPES:
        par[f"xT_{nt}"] = nc.declare_dram_parameter(
            f"xT_{nt}", [cfg["MOD"][nt], shard[nt]], mybir.dt.bfloat16, False)
        par[f"We_{nt}"] = nc.declare_dram_parameter(
            f"We_{nt}", [cfg["MOD"][nt], cfg["D_IN"]], mybir.dt.bfloat16, False)
        par[f"be_{nt}"] = nc.declare_dram_parameter(
            f"be_{nt}", [cfg["D_IN"], 1], mybir.dt.float32, False)
    for l in range(3):
        od = DOUT if l == 2 else DH
        par[f"W{l}"] = nc.declare_dram_parameter(
            f"W{l}", [NREL, DH, od], mybir.dt.bfloat16, False)
        par[f"L{l}"] = nc.declare_dram_parameter(
            f"L{l}", [DH, od], mybir.dt.bfloat16, False)
        par[f"b{l}"] = nc.declare_dram_parameter(
            f"b{l}", [od, 1], mybir.dt.float32, False)
    for R in S["rels"]:
        tg = R["tag"]
        par[f"idx_{tg}"] = nc.declare_dram_parameter(
            f"idx_{tg}", [P, R["nslots"] // 16], mybir.dt.int16, False)
        par[f"seg_{tg}"] = nc.declare_dram_parameter(
            f"seg_{tg}", [P, R["nblk_tot"], P], mybir.dt.float8e4, False)
        par[f"invd_{tg}"] = nc.declare_dram_parameter(
            f"invd_{tg}", [P, R["NW"]], mybir.dt.float32, False)
    out_par = nc.declare_dram_parameter("out", [nsh_tot, DOUT],
                                        mybir.dt.float32, True)

    agin, tabs = {}, {}
    for l in range(3):
        for nt in SRC_NTYPES:
            agin[(l, nt)] = nc.dram_tensor(
                f"agin{l}_{nt}", [shard[nt], DH], mybir.dt.bfloat16)
            tabs[(l, nt)] = nc.dram_tensor(
                f"tab{l}_{nt}", [cfg["N"][nt] + 2, DH], mybir.dt.bfloat16,
                addr_space="Shared")

    with ExitStack() as ctx:
        tc = ctx.enter_context(tile.TileContext(nc))
        nc.gpsimd.load_library(library_config.mlp)

        const = ctx.enter_context(tc.tile_pool(name="const", bufs=1))
        persist = ctx.enter_context(tc.tile_pool(name="persist", bufs=1))
        gpool = ctx.enter_context(tc.tile_pool(name="gpool", bufs=12))
        ipool = ctx.enter_context(tc.tile_pool(name="ipool", bufs=10))
        xpool = ctx.enter_context(tc.tile_pool(name="xpool", bufs=2))
        wpool = ctx.enter_context(tc.tile_pool(name="wpool", bufs=4))
        spool = ctx.enter_context(tc.tile_pool(name="spool", bufs=12))
        pst = ctx.enter_context(tc.tile_pool(name="pst", bufs=2, space="PSUM"))
        psA = ctx.enter_context(tc.tile_pool(name="psA", bufs=2, space="PSUM"))
        psB = ctx.enter_context(tc.tile_pool(name="psB", bufs=2, space="PSUM"))
        psE = ctx.enter_context(tc.tile_pool(name="psE", bufs=2, space="PSUM"))

        identity = const.tile([P, P], mybir.dt.float32)
        from concourse.masks import make_identity
        make_identity(nc, identity[:])
        identity16 = const.tile([P, P], mybir.dt.bfloat16)
        nc.vector.tensor_copy(identity16[:], identity[:])

        sb_W, sb_L, sb_b = {}, {}, {}
        for l in range(3):
            od = DOUT if l == 2 else DH
            t = const.tile([DH, NREL, od], mybir.dt.bfloat16, tag=f"W{l}")
            nc.sync.dma_start(t[:], par[f"W{l}"][:].rearrange("r k o -> k r o"))
            sb_W[l] = t
            sb_L[l] = const.tile([DH, od], mybir.dt.bfloat16, tag=f"L{l}",
                                 name=f"L{l}")
            nc.sync.dma_start(sb_L[l][:], par[f"L{l}"][:])
            sb_b[l] = const.tile([od, 1], mybir.dt.float32, tag=f"b{l}",
                                 name=f"b{l}")
            nc.sync.dma_start(sb_b[l][:], par[f"b{l}"][:])

        sb_invd = {}
        for R in S["rels"]:
            tg = R["tag"]
            t = const.tile([P, R["NW"]], mybir.dt.float32, tag=f"invd_{tg}",
                           name=f"invd_{tg}")
            nc.sync.dma_start(t[:], par[f"invd_{tg}"][:])
            sb_invd[tg] = t

        zrow = const.tile([1, DH], mybir.dt.bfloat16)
        nc.vector.memset(zrow[:], 0.0)
        for l in range(3):
            for nt in SRC_NTYPES:
                n = cfg["N"][nt]
                nc.sync.dma_start(tabs[(l, nt)][0:1, :], zrow[:])
                nc.sync.dma_start(tabs[(l, nt)][n + 1:n + 2, :], zrow[:])

        # zero the gather buffers once: trailing-pad descriptors are trimmed
        # (idx -1), so untouched slots must hold finite bf16 (0 x seg-zero).
        for _ in range(12):
            g0 = gpool.tile([P, CHUNK, P], mybir.dt.bfloat16, tag="gat")
            nc.vector.memset(g0[:], 0.0)

        hT = [persist.tile([DH, nsh_tot], mybir.dt.bfloat16, tag=f"hT{i}",
                           name=f"hT{i}")
              for i in range(2)]
        nt_off, o = {}, 0
        for nt in NTYPES:
            nt_off[nt] = o
            o += shard[nt]
        praw = persist.tile([DH, praw_cols], mybir.dt.bfloat16, tag="praw")

        gq_counter = [0]

        def emit_embedding(nt):
            mod, sh = cfg["MOD"][nt], shard[nt]
            kt = mod // P
            sb_we = xpool.tile([P, 8, cfg["D_IN"]], mybir.dt.bfloat16, tag="we")
            nc.sync.dma_start(
                sb_we[:, :kt, :],
                par[f"We_{nt}"][:].rearrange("(k p) f -> p k f", p=P))
            sb_be = wpool.tile([cfg["D_IN"], 1], mybir.dt.float32, tag="be")
            nc.sync.dma_start(sb_be[:], par[f"be_{nt}"][:])
            for n0 in range(0, sh, 512):
                n1 = min(n0 + 512, sh)
                cols = n1 - n0
                xt = xpool.tile([P, 8, 512], mybir.dt.bfloat16, tag="xt")
                nc.sync.dma_start(
                    xt[:, :kt, :cols],
                    par[f"xT_{nt}"][:].rearrange(
                        "(k p) n -> p k n", p=P)[:, :, n0:n1])
                pe = psE.tile([P, 512], mybir.dt.float32, tag="emb")
                for k in range(kt):
                    nc.tensor.matmul(pe[:, :cols], sb_we[:, k, :],
                                     xt[:, k, :cols],
                                     start=(k == 0), stop=(k == kt - 1))
                nc.scalar.activation(
                    hT[0][:, nt_off[nt] + n0:nt_off[nt] + n1], pe[:, :cols],
                    mybir.ActivationFunctionType.Identity, bias=sb_be[:])

        def stage_ag_window(l, nt, w0, cols):
            """Transpose one hT[l] window of ntype nt into the AllGather
            staging buffer."""
            src = hT[l % 2][:, nt_off[nt] + w0:nt_off[nt] + w0 + cols]
            pt = pst.tile([P, P], mybir.dt.bfloat16, tag="tp", name="pt16")
            nc.tensor.transpose(pt[:cols, :DH], src, identity16[:])
            stg = wpool.tile([P, DH], mybir.dt.bfloat16, tag="agstg")
            nc.vector.tensor_copy(stg[:cols, :], pt[:cols, :DH])
            nc.sync.dma_start(agin[(l, nt)][w0:w0 + cols, :], stg[:cols, :])

        def emit_ag_collective(l, nt):
            nc.gpsimd.collective_compute(
                "AllGather", mybir.AluOpType.bypass,
                replica_groups=[list(range(ncore))],
                ins=[agin[(l, nt)][:]],
                outs=[tabs[(l, nt)][1:cfg["N"][nt] + 1]],
            )

        def emit_ag(l, nt):
            for w0 in range(0, shard[nt], P):
                stage_ag_window(l, nt, w0, min(P, shard[nt] - w0))
            emit_ag_collective(l, nt)

        def emit_window(l, dnt, tags, w):
            """One dst window: per-relation gathers + seg matmuls into PSUM
            agg -> praw, then W_r matmuls + self-loop + activation."""
            od = DOUT if l == 2 else DH
            sh = shard[dnt]
            cs = nt_off[dnt] + w * P
            ce = min(cs + P, nt_off[dnt] + sh)
            cols = ce - cs
            live = []
            for tg in tags:
                R = rel_by_tag[tg]
                gs = [g for g in R["gathers"] if g[0] == w]
                nmm = sum(g[4] for g in gs)
                if nmm == 0:
                    nc.vector.memset(
                        praw[:, praw_off[tg] + w * P:
                         praw_off[tg] + (w + 1) * P], 0.0)
                    continue
                pa = psA.tile([P, P], mybir.dt.float32, tag="agg")
                mm = 0
                for (_, b, soff, slots, nb, segoff) in gs:
                    sbi = ipool.tile([P, maxblk_all * P // 16],
                                     mybir.dt.int16, tag="idx")
                    nc.sync.dma_start(
                        sbi[:, :slots // 16],
                        par[f"idx_{tg}"][:, soff // 16:(soff + slots) // 16])
                    b0, b1, _ = R["banks"][b]
                    for k0 in range(0, nb, CHUNK):
                        nbc = min(CHUNK, nb - k0)
                        csl = nbc * P
                        gt = gpool.tile([P, CHUNK, P], mybir.dt.bfloat16,
                                        tag="gat")
                        q = gq_counter[0] % NQ
                        gq_counter[0] += 1
                        nc.gpsimd.dma_gather(
                            out_ap=gt[:, :nbc, :],
                            in_ap=tabs[(l, R["snt"])][b0:b1],
                            idxs_ap=sbi[:, k0 * 8:k0 * 8 + csl // 16],
                            num_idxs=csl, num_idxs_reg=csl,
                            elem_size=DH, transpose=False, single_packet=True,
                            queue_num=q)
                        sg = spool.tile([P, CHUNK, P], mybir.dt.float8e4,
                                        tag="sg")
                        nc.sync.dma_start(
                            sg[:, :nbc, :],
                            par[f"seg_{tg}"][:, segoff + k0:segoff + k0 + nbc, :])
                        for k in range(nbc):
                            # dst-major: pa[dst, feat] = seg^T @ gathered
                            nc.tensor.matmul(pa[:, :], sg[:, k, :], gt[:, k, :],
                                             start=(mm == 0),
                                             stop=(mm == nmm - 1))
                            mm += 1
                live.append((tg, pa))
            for tg, pa in live:
                # inv_deg as per-dst-partition scale while copying to SBUF,
                # then PE-transpose back to the feat-major praw layout
                tr = wpool.tile([P, P], mybir.dt.bfloat16, tag="prawT")
                with nc.allow_low_precision(reason="praw is consumed by a "
                                            "bf16 matmul"):
                    nc.scalar.activation(
                        tr[:cols, :], pa[:cols, :],
                        mybir.ActivationFunctionType.Identity,
                        scale=sb_invd[tg][:cols, w:w + 1])
                pt = pst.tile([P, P], mybir.dt.bfloat16, tag="tp",
                              name="pt16")
                nc.tensor.transpose(pt[:, :cols], tr[:cols, :],
                                    identity16[:cols, :cols])
                nc.vector.tensor_copy(
                    praw[:, praw_off[tg] + w * P:
                         praw_off[tg] + w * P + cols], pt[:, :cols])
            pb = psB.tile([P, P], mybir.dt.float32, tag="out2")
            for ti, tg in enumerate(tags):
                R = rel_by_tag[tg]
                nc.tensor.matmul(
                    pb[:od, :cols], sb_W[l][:, R["r"], :],
                    praw[:, praw_off[tg] + w * P:praw_off[tg] + w * P + cols],
                    start=(ti == 0), stop=False)
            nc.tensor.matmul(pb[:od, :cols], sb_L[l][:], hT[l % 2][:, cs:ce],
                             start=False, stop=True)
            if l < 2:
                nc.scalar.activation(
                    hT[(l + 1) % 2][:od, cs:ce], pb[:od, :cols],
                    mybir.ActivationFunctionType.Relu, bias=sb_b[l][:])
                if dnt in SRC_NTYPES:
                    stage_ag_window(l + 1, dnt, w * P, cols)
            else:
                fin = wpool.tile([P, P], mybir.dt.float32, tag="fin")
                nc.scalar.activation(
                    fin[:od, :cols], pb[:od, :cols],
                    mybir.ActivationFunctionType.Identity, bias=sb_b[l][:])
                pt = pst.tile([P, P], mybir.dt.float32, tag="tp")
                nc.tensor.transpose(pt[:cols, :od], fin[:od, :cols],
                                    identity[:od, :od])
                stg = wpool.tile([P, DOUT], mybir.dt.float32, tag="ostg")
                nc.vector.tensor_copy(stg[:cols, :], pt[:cols, :od])
                nc.sync.dma_start(out_par[cs:ce, :], stg[:cols, :])

        def emit_phase(l, dnt, tags):
            for w in range(nw[dnt]):
                emit_window(l, dnt, tags, w)

        # ---- program ----
        # Each window's activation stages its next-layer AllGather input;
        # the collective for an ntype is dispatched right after that ntype's
        # windows, so its wire time overlaps the remaining phases' gathers.
        emit_embedding("drug")
        emit_ag(0, "drug")
        emit_embedding("gene")
        emit_ag(0, "gene")
        emit_embedding("disease")
        for l in range(3):
            emit_phase(l, *PHASES[0])
            if l < 2:
                emit_ag_collective(l + 1, "drug")
            emit_phase(l, *PHASES[1])
            if l < 2:
                emit_ag_collective(l + 1, "gene")
            emit_phase(l, *PHASES[2])

    nc.compile()
    return nc


# ---------------------------------------------------------------------------
# entry point
# ---------------------------------------------------------------------------

def _install_ntff_hook():
    if "antenv.axon_hooks" in sys.modules:
        return
    mod = types.ModuleType("antenv.axon_hooks")
    mod._hook = None
    mod.set_axon_ntff_profile_hook = lambda h: setattr(mod, "_hook", h)
    mod.get_axon_ntff_profile_hook = lambda: mod._hook
    sys.modules["antenv.axon_hooks"] = mod
    try:
        import antenv
        antenv.axon_hooks = mod
        from trn_agent_boot.trn_boot import _ntff_profile_via_ctypes
        hook = _ntff_profile_via_ctypes("/opt/axon/libaxon_pjrt.so")
        if hook is not None:
            mod.set_axon_ntff_profile_hook(hook)
    except Exception:
        pass


def run(inputs, cfg=CFG, trace=False, tmpdir=None):
    S, percore = preprocess(cfg, inputs)
    nc = build(S)
    _install_ntff_hook()
    from concourse import bass_utils
    bass_utils.upload_artifacts = lambda d: d
    res = bass_utils.run_bass_kernel_spmd(
        nc, percore, list(range(cfg["NCORE"])), trace=trace, tmpdir=tmpdir,
        trace_cores=[0] if trace else None)
    ncore = cfg["NCORE"]
    shard = {nt: cfg["N"][nt] // ncore for nt in NTYPES}
    outs = []
    o = 0
    for nt in NTYPES:
        parts = [res.results[c]["out"][o:o + shard[nt]] for c in range(ncore)]
        outs.append(np.concatenate(parts, 0))
        o += shard[nt]
    full = np.concatenate(outs, 0).astype(np.float32)
    run.last_exec_time_ns = res.exec_time_ns
    return full


def kernel(**inputs):
    return run(inputs)



# revision 19
# speedup vs baseline: 1.2468x; 1.0208x over previous
"""Trainium2 Bass kernel for nn_BaseRGCNHetero (3-layer heterogeneous RGCN).

Strategy (8 NeuronCores, SPMD):
  - Destination-shard the nodes: core c owns rows [c*N/8, (c+1)*N/8) of every
    node type; all edges whose dst is in the shard are processed there, so
    per-relation aggregates need no cross-core reduction.
  - Aggregate-first algebra: agg[dst] = (sum_{e->dst} h[src]) @ W_r * inv_deg,
    sharing one bf16 DRAM gather table per source ntype (drug, gene) per
    layer.  After each layer the drug/gene h-shards are AllGathered (bf16)
    into the next layer's tables; each AllGather is issued as soon as its
    ntype's windows finish so the wire time overlaps the remaining gathers.
  - Edge slot stream per relation: 128-dst windows in natural order, one
    chunk per (window, src bank), slots sorted by src row and padded to a
    multiple of 128 (pad slots point at an all-zero table row).  Non-
    transposed dma_gather pulls h[src] rows node-major ([slot, feat] blocks);
    gathers round-robin over 4 SWDGE queues so descriptor generation runs on
    all four GpSimd Q7 core pairs concurrently (3.3x single-queue, and the
    non-transpose path avoids the xbar that makes concurrent transposed
    gathers corrupt each other).
  - Segment sums on TensorE: per 128-slot block, VectorE builds a one-hot
    seg matrix seg[slot, dst] = (dstcol[slot] == dst) * inv_deg[slot] from
    host-streamed per-block columns; matmul(gt_block^T @ seg) accumulates
    agg[feat, dst] for the window in PSUM across the window's blocks.
  - Per dst window: agg -> bf16 praw, then one matmul per relation (W_r) plus
    the self-loop h @ L accumulate in a single PSUM bank; bias (+relu) is a
    fused ScalarE activation per window.
"""
import os
import sys
import types
import numpy as np
import ml_dtypes
from contextlib import ExitStack

# Fund a larger SWDGE descriptor carveout (ring depth) so gather desc-gen can
# run ahead of the DMA drain; must match Bacc's dynamic_dma_scratch_size.
os.environ.setdefault("TRNINF_DYNAMIC_DMA_SCRATCH_SIZE", "32768")



import concourse.bass as bass
import concourse.bacc as bacc
import concourse.tile as tile
from concourse import mybir, library_config

BF16 = ml_dtypes.bfloat16
FP8 = ml_dtypes.float8_e4m3
P = 128
NQ = 4             # SWDGE queues (gather descriptor-gen parallelism)
CHUNK = 4          # gather call granularity in 128-slot blocks (512 descs =
                   # half a ring, so the ring holds two chunks and desc-gen
                   # pipelines ahead of the drain; chunks round-robin queues
                   # so all 4 rings drain concurrently)

CFG = dict(
    N={"drug": 20000, "gene": 50000, "disease": 10000},
    MOD={"drug": 1024, "gene": 768, "disease": 512},
    D_IN=128, D_H=128, D_OUT=64,
    RELS=[("drug", "disease", "dd"), ("drug", "drug", "ddr"),
          ("drug", "gene", "dg"), ("gene", "disease", "gd"),
          ("gene", "gene", "gg")],
    NCORE=8,
    BANK=32768,     # dma_gather int16 row-index limit per table slice
)

NTYPES = ("drug", "gene", "disease")
SRC_NTYPES = ("drug", "gene")
# layer processing phases: dst ntype -> relations feeding it (tags).
# drug first: its AllGather (issued right after the drug windows) then hides
# under the gene+disease phases; the gene AllGather hides under disease.
PHASES = [("drug", ["ddr"]), ("gene", ["dg", "gg"]), ("disease", ["dd", "gd"])]


# ---------------------------------------------------------------------------
# host-side preprocessing
# ---------------------------------------------------------------------------

def _pack_idx(stream):
    """int array (len % 128 == 0) -> dma_gather idx layout [128, len/16] int16:
    idx i at (i%16, i//16), replicated across the 8 groups of 16 partitions."""
    n = stream.size
    v = stream.astype(np.int16).reshape(n // 16, 16).T
    return np.tile(v, (8, 1))


def _banks(cfg, snt):
    """Gather-table bank slices for source ntype snt.
    Table rows: 0 = zeros, 1..N = nodes, N+1 = zeros.
    Returns list of (start_row, end_row, pad_row_relative)."""
    n = cfg["N"][snt]
    trows = n + 2
    if trows <= cfg["BANK"]:
        return [(0, trows, 0)]
    return [(0, cfg["BANK"], 0), (cfg["BANK"], trows, n + 1 - cfg["BANK"])]


def preprocess(cfg, inputs):
    ncore = cfg["NCORE"]
    shard = {nt: cfg["N"][nt] // ncore for nt in NTYPES}
    nw = {nt: -(-shard[nt] // P) for nt in NTYPES}

    S = dict(cfg=cfg, nw=nw, shard=shard, rels=[])
    percore = [dict() for _ in range(ncore)]

    for r, (snt, dnt, tag) in enumerate(cfg["RELS"]):
        src = np.asarray(inputs["e_" + tag + "_s"]).astype(np.int64)
        dst = np.asarray(inputs["e_" + tag + "_d"]).astype(np.int64)
        banks = _banks(cfg, snt)
        nbank = len(banks)
        NW = nw[dnt]
        dsh = shard[dnt]

        core_of = dst // dsh
        deg_all = np.bincount(dst, minlength=cfg["N"][dnt]).astype(np.float32)
        inv_deg = 1.0 / np.maximum(deg_all, 1.0)

        row_all = src + 1
        bank_of = (row_all >= cfg["BANK"]).astype(np.int64) if nbank == 2 \
            else np.zeros(row_all.size, np.int64)

        # per-core per-(window, bank) UNIQUE-src counts -> shared block counts
        # (duplicate srcs within a (window, bank) share one gathered slot; the
        # seg row then carries several nonzeros)
        cnt = np.zeros((ncore, NW, nbank), np.int64)
        ld_all = dst - core_of * dsh
        w_all = ld_all // P
        for c in range(ncore):
            m = core_of == c
            ukey = (w_all[m] * nbank + bank_of[m]) * 65536 + row_all[m]
            key = np.unique(ukey) // 65536
            cnt[c] = np.bincount(key, minlength=NW * nbank).reshape(NW, nbank)
        nblk = -(-cnt.max(axis=0) // P)          # [NW, nbank] shared
        slots_wb = nblk * P
        off_wb = np.zeros((NW, nbank), np.int64)
        gathers = []                              # (bank, off, slots, nblk, segoff)
        off = 0
        segoff = 0
        for w in range(NW):
            for b in range(nbank):
                if nblk[w, b] == 0:
                    continue
                off_wb[w, b] = off
                gathers.append((w, b, int(off), int(slots_wb[w, b]),
                                int(nblk[w, b]), int(segoff)))
                off += int(slots_wb[w, b])
                segoff += int(nblk[w, b])
        nslots = max(off, P)
        nblk_tot = max(segoff, 1)
        maxblk = int(nblk.max()) if nblk.size else 1

        for c in range(ncore):
            stream = np.zeros(nslots, np.int16)
            segm = np.zeros((nblk_tot, P, P), np.float32)
            for w, b, o, sl, nb, so in gathers:
                stream[o:o + sl] = banks[b][2]
            m = core_of == c
            e_row = row_all[m] - np.array([bk[0] for bk in banks])[bank_of[m]]
            e_b = bank_of[m]
            e_ld = ld_all[m]
            e_w = e_ld // P
            e_dl = e_ld % P
            order = np.lexsort((e_row, e_b, e_w))
            key = (e_w * nbank + e_b)[order]
            starts = np.r_[0, np.flatnonzero(np.diff(key)) + 1]
            sizes = np.diff(np.r_[starts, key.size])
            # dedup: equal (w, b, row) edges share a slot
            okey = key * 65536 + e_row[order]
            new = np.r_[True, np.diff(okey) != 0]
            cr = np.cumsum(new) - 1
            urank = cr - np.repeat(cr[starts], sizes)
            pos = off_wb[e_w[order], e_b[order]] + urank
            stream[pos] = e_row[order].astype(np.int16)
            # seg[slot, dst] 0/1/2... counts (small ints, exact in fp8);
            # inv_deg is applied on-device as a per-dst-partition scale after
            # the swapped (dst-major) segment matmul.
            np.add.at(segm, (pos // P, pos % P, e_dl[order]), 1.0)
            percore[c][f"idx_{tag}"] = _pack_idx(stream)
            percore[c][f"seg_{tag}"] = np.ascontiguousarray(
                segm.transpose(1, 0, 2)).astype(FP8)
            # per-window inv_deg columns for this relation: invd[dl, w]
            dsh0 = c * dsh
            iv = np.ones((NW * P,), np.float32)
            nloc = min(dsh, cfg["N"][dnt] - dsh0)
            iv[:nloc] = inv_deg[dsh0:dsh0 + nloc]
            percore[c][f"invd_{tag}"] = np.ascontiguousarray(
                iv.reshape(NW, P).T)

        S["rels"].append(dict(r=r, snt=snt, dnt=dnt, tag=tag, NW=NW,
                              banks=banks, gathers=gathers, nslots=nslots,
                              nblk_tot=nblk_tot, maxblk=maxblk))

    for nt in NTYPES:
        x = np.asarray(inputs["x_" + nt])
        for c in range(ncore):
            sh = shard[nt]
            percore[c][f"xT_{nt}"] = np.ascontiguousarray(
                x[c * sh:(c + 1) * sh].T).astype(BF16)

    com = dict()
    for nt in NTYPES:
        com[f"We_{nt}"] = np.asarray(inputs["We_" + nt]).astype(BF16)
        com[f"be_{nt}"] = np.asarray(inputs["be_" + nt]).astype(
            np.float32).reshape(-1, 1)
    for l in range(3):
        com[f"W{l}"] = np.asarray(inputs[f"W{l}"]).astype(BF16)
        com[f"L{l}"] = np.asarray(inputs[f"L{l}"]).astype(BF16)
        com[f"b{l}"] = np.asarray(inputs[f"b{l}"]).astype(np.float32).reshape(-1, 1)
    for c in range(ncore):
        percore[c].update(com)
    return S, percore


# ---------------------------------------------------------------------------
# device program
# ---------------------------------------------------------------------------

def build(S):
    cfg = S["cfg"]
    ncore = cfg["NCORE"]
    nw, shard = S["nw"], S["shard"]
    DH, DOUT = cfg["D_H"], cfg["D_OUT"]
    NREL = len(cfg["RELS"])
    nsh_tot = sum(shard.values())
    rel_by_tag = {R["tag"]: R for R in S["rels"]}
    maxblk_all = max(R["maxblk"] for R in S["rels"])
    # praw16 column offsets: every relation gets its own region so phases
    # never alias (a shared buffer serializes phases on tile WAR hazards)
    praw_off = {}
    praw_cols = 0
    for dnt, tags in PHASES:
        for tg in tags:
            praw_off[tg] = praw_cols
            praw_cols += nw[dnt] * P

    nc = bacc.Bacc("TRN2", target_bir_lowering=False, debug=False,
                   num_devices=ncore, num_swdge_queues=NQ,
                   dynamic_dma_scratch_size=32768)

    par = {}
    for nt in NTYPES:
        par[f"xT_{nt}"] = nc.declare_dram_parameter(
            f"xT_{nt}", [cfg["MOD"][nt], shard[nt]], mybir.dt.bfloat16, False)
        par[f"We_{nt}"] = nc.declare_dram_parameter(
            f"We_{nt}", [cfg["MOD"][nt], cfg["D_IN"]], mybir.dt.bfloat16, False)
        par[f"be_{nt}"] = nc.declare_dram_parameter(
            f"be_{nt}", [cfg["D_IN"], 1], mybir.dt.float32, False)
    for l in range(3):
        od = DOUT if l == 2 else DH
        par[f"W{l}"] = nc.declare_dram_parameter(
            f"W{l}", [NREL, DH, od], mybir.dt.bfloat16, False)
        par[f"L{l}"] = nc.declare_dram_parameter(
            f"L{l}", [DH, od], mybir.dt.bfloat16, False)
        par[f"b{l}"] = nc.declare_dram_parameter(
            f"b{l}", [od, 1], mybir.dt.float32, False)
    for R in S["rels"]:
        tg = R["tag"]
        par[f"idx_{tg}"] = nc.declare_dram_parameter(
            f"idx_{tg}", [P, R["nslots"] // 16], mybir.dt.int16, False)
        par[f"seg_{tg}"] = nc.declare_dram_parameter(
            f"seg_{tg}", [P, R["nblk_tot"], P], mybir.dt.float8e4, False)
        par[f"invd_{tg}"] = nc.declare_dram_parameter(
            f"invd_{tg}", [P, R["NW"]], mybir.dt.float32, False)
    out_par = nc.declare_dram_parameter("out", [nsh_tot, DOUT],
                                        mybir.dt.float32, True)

    agin, tabs = {}, {}
    for l in range(3):
        for nt in SRC_NTYPES:
            agin[(l, nt)] = nc.dram_tensor(
                f"agin{l}_{nt}", [shard[nt], DH], mybir.dt.bfloat16)
            tabs[(l, nt)] = nc.dram_tensor(
                f"tab{l}_{nt}", [cfg["N"][nt] + 2, DH], mybir.dt.bfloat16,
                addr_space="Shared")

    with ExitStack() as ctx:
        tc = ctx.enter_context(tile.TileContext(nc))
        nc.gpsimd.load_library(library_config.mlp)

        const = ctx.enter_context(tc.tile_pool(name="const", bufs=1))
        persist = ctx.enter_context(tc.tile_pool(name="persist", bufs=1))
        gpool = ctx.enter_context(tc.tile_pool(name="gpool", bufs=12))
        ipool = ctx.enter_context(tc.tile_pool(name="ipool", bufs=10))
        xpool = ctx.enter_context(tc.tile_pool(name="xpool", bufs=2))
        wpool = ctx.enter_context(tc.tile_pool(name="wpool", bufs=4))
        spool = ctx.enter_context(tc.tile_pool(name="spool", bufs=12))
        pst = ctx.enter_context(tc.tile_pool(name="pst", bufs=2, space="PSUM"))
        psA = ctx.enter_context(tc.tile_pool(name="psA", bufs=2, space="PSUM"))
        psB = ctx.enter_context(tc.tile_pool(name="psB", bufs=2, space="PSUM"))
        psE = ctx.enter_context(tc.tile_pool(name="psE", bufs=2, space="PSUM"))

        identity = const.tile([P, P], mybir.dt.float32)
        from concourse.masks import make_identity
        make_identity(nc, identity[:])
        identity16 = const.tile([P, P], mybir.dt.bfloat16)
        nc.vector.tensor_copy(identity16[:], identity[:])

        sb_W, sb_L, sb_b = {}, {}, {}
        for l in range(3):
            od = DOUT if l == 2 else DH
            t = const.tile([DH, NREL, od], mybir.dt.bfloat16, tag=f"W{l}")
            nc.sync.dma_start(t[:], par[f"W{l}"][:].rearrange("r k o -> k r o"))
            sb_W[l] = t
            sb_L[l] = const.tile([DH, od], mybir.dt.bfloat16, tag=f"L{l}",
                                 name=f"L{l}")
            nc.sync.dma_start(sb_L[l][:], par[f"L{l}"][:])
            sb_b[l] = const.tile([od, 1], mybir.dt.float32, tag=f"b{l}",
                                 name=f"b{l}")
            nc.sync.dma_start(sb_b[l][:], par[f"b{l}"][:])

        sb_invd = {}
        for R in S["rels"]:
            tg = R["tag"]
            t = const.tile([P, R["NW"]], mybir.dt.float32, tag=f"invd_{tg}",
                           name=f"invd_{tg}")
            nc.sync.dma_start(t[:], par[f"invd_{tg}"][:])
            sb_invd[tg] = t

        zrow = const.tile([1, DH], mybir.dt.bfloat16)
        nc.vector.memset(zrow[:], 0.0)
        for l in range(3):
            for nt in SRC_NTYPES:
                n = cfg["N"][nt]
                nc.sync.dma_start(tabs[(l, nt)][0:1, :], zrow[:])
                nc.sync.dma_start(tabs[(l, nt)][n + 1:n + 2, :], zrow[:])

        # zero the gather buffers once: trailing-pad descriptors are trimmed
        # (idx -1), so untouched slots must hold finite bf16 (0 x seg-zero).
        for _ in range(12):
            g0 = gpool.tile([P, CHUNK, P], mybir.dt.bfloat16, tag="gat")
            nc.vector.memset(g0[:], 0.0)

        hT = [persist.tile([DH, nsh_tot], mybir.dt.bfloat16, tag=f"hT{i}",
                           name=f"hT{i}")
              for i in range(2)]
        nt_off, o = {}, 0
        for nt in NTYPES:
            nt_off[nt] = o
            o += shard[nt]
        praw = persist.tile([DH, praw_cols], mybir.dt.bfloat16, tag="praw")

        gq_counter = [0]

        def emit_embedding(nt):
            mod, sh = cfg["MOD"][nt], shard[nt]
            kt = mod // P
            sb_we = xpool.tile([P, 8, cfg["D_IN"]], mybir.dt.bfloat16, tag="we")
            nc.sync.dma_start(
                sb_we[:, :kt, :],
                par[f"We_{nt}"][:].rearrange("(k p) f -> p k f", p=P))
            sb_be = wpool.tile([cfg["D_IN"], 1], mybir.dt.float32, tag="be")
            nc.sync.dma_start(sb_be[:], par[f"be_{nt}"][:])
            for n0 in range(0, sh, 512):
                n1 = min(n0 + 512, sh)
                cols = n1 - n0
                xt = xpool.tile([P, 8, 512], mybir.dt.bfloat16, tag="xt")
                nc.sync.dma_start(
                    xt[:, :kt, :cols],
                    par[f"xT_{nt}"][:].rearrange(
                        "(k p) n -> p k n", p=P)[:, :, n0:n1])
                pe = psE.tile([P, 512], mybir.dt.float32, tag="emb")
                for k in range(kt):
                    nc.tensor.matmul(pe[:, :cols], sb_we[:, k, :],
                                     xt[:, k, :cols],
                                     start=(k == 0), stop=(k == kt - 1))
                nc.scalar.activation(
                    hT[0][:, nt_off[nt] + n0:nt_off[nt] + n1], pe[:, :cols],
                    mybir.ActivationFunctionType.Identity, bias=sb_be[:])

        def stage_ag_window(l, nt, w0, cols):
            """Transpose one hT[l] window of ntype nt into the AllGather
            staging buffer."""
            src = hT[l % 2][:, nt_off[nt] + w0:nt_off[nt] + w0 + cols]
            pt = pst.tile([P, P], mybir.dt.bfloat16, tag="tp", name="pt16")
            nc.tensor.transpose(pt[:cols, :DH], src, identity16[:])
            stg = wpool.tile([P, DH], mybir.dt.bfloat16, tag="agstg")
            nc.vector.tensor_copy(stg[:cols, :], pt[:cols, :DH])
            nc.sync.dma_start(agin[(l, nt)][w0:w0 + cols, :], stg[:cols, :])

        def emit_ag_collective(l, nt):
            nc.gpsimd.collective_compute(
                "AllGather", mybir.AluOpType.bypass,
                replica_groups=[list(range(ncore))],
                ins=[agin[(l, nt)][:]],
                outs=[tabs[(l, nt)][1:cfg["N"][nt] + 1]],
            )

        def emit_ag(l, nt):
            for w0 in range(0, shard[nt], P):
                stage_ag_window(l, nt, w0, min(P, shard[nt] - w0))
            emit_ag_collective(l, nt)

        def emit_window(l, dnt, tags, w):
            """One dst window: per-relation gathers + seg matmuls into PSUM
            agg -> praw, then W_r matmuls + self-loop + activation."""
            od = DOUT if l == 2 else DH
            sh = shard[dnt]
            cs = nt_off[dnt] + w * P
            ce = min(cs + P, nt_off[dnt] + sh)
            cols = ce - cs
            live = []
            for tg in tags:
                R = rel_by_tag[tg]
                gs = [g for g in R["gathers"] if g[0] == w]
                nmm = sum(g[4] for g in gs)
                if nmm == 0:
                    nc.vector.memset(
                        praw[:, praw_off[tg] + w * P:
                         praw_off[tg] + (w + 1) * P], 0.0)
                    continue
                pa = psA.tile([P, P], mybir.dt.float32, tag="agg")
                mm = 0
                for (_, b, soff, slots, nb, segoff) in gs:
                    sbi = ipool.tile([P, maxblk_all * P // 16],
                                     mybir.dt.int16, tag="idx")
                    nc.sync.dma_start(
                        sbi[:, :slots // 16],
                        par[f"idx_{tg}"][:, soff // 16:(soff + slots) // 16])
                    b0, b1, _ = R["banks"][b]
                    for k0 in range(0, nb, CHUNK):
                        nbc = min(CHUNK, nb - k0)
                        csl = nbc * P
                        gt = gpool.tile([P, CHUNK, P], mybir.dt.bfloat16,
                                        tag="gat")
                        q = gq_counter[0] % NQ
                        gq_counter[0] += 1
                        nc.gpsimd.dma_gather(
                            out_ap=gt[:, :nbc, :],
                            in_ap=tabs[(l, R["snt"])][b0:b1],
                            idxs_ap=sbi[:, k0 * 8:k0 * 8 + csl // 16],
                            num_idxs=csl, num_idxs_reg=csl,
                            elem_size=DH, transpose=False, single_packet=True,
                            queue_num=q)
                        sg = spool.tile([P, CHUNK, P], mybir.dt.float8e4,
                                        tag="sg")
                        nc.sync.dma_start(
                            sg[:, :nbc, :],
                            par[f"seg_{tg}"][:, segoff + k0:segoff + k0 + nbc, :])
                        for k in range(nbc):
                            # dst-major: pa[dst, feat] = seg^T @ gathered
                            nc.tensor.matmul(pa[:, :], sg[:, k, :], gt[:, k, :],
                                             start=(mm == 0),
                                             stop=(mm == nmm - 1))
                            mm += 1
                live.append((tg, pa))
            for tg, pa in live:
                # inv_deg as per-dst-partition scale while copying to SBUF,
                # then PE-transpose back to the feat-major praw layout
                tr = wpool.tile([P, P], mybir.dt.bfloat16, tag="prawT")
                with nc.allow_low_precision(reason="praw is consumed by a "
                                            "bf16 matmul"):
                    nc.scalar.activation(
                        tr[:cols, :], pa[:cols, :],
                        mybir.ActivationFunctionType.Identity,
                        scale=sb_invd[tg][:cols, w:w + 1])
                pt = pst.tile([P, P], mybir.dt.bfloat16, tag="tp",
                              name="pt16")
                nc.tensor.transpose(pt[:, :cols], tr[:cols, :],
                                    identity16[:cols, :cols])
                nc.vector.tensor_copy(
                    praw[:, praw_off[tg] + w * P:
                         praw_off[tg] + w * P + cols], pt[:, :cols])
            pb = psB.tile([P, P], mybir.dt.float32, tag="out2")
            for ti, tg in enumerate(tags):
                R = rel_by_tag[tg]
                nc.tensor.matmul(
                    pb[:od, :cols], sb_W[l][:, R["r"], :],
                    praw[:, praw_off[tg] + w * P:praw_off[tg] + w * P + cols],
                    start=(ti == 0), stop=False)
            nc.tensor.matmul(pb[:od, :cols], sb_L[l][:], hT[l % 2][:, cs:ce],
                             start=False, stop=True)
            if l < 2:
                nc.scalar.activation(
                    hT[(l + 1) % 2][:od, cs:ce], pb[:od, :cols],
                    mybir.ActivationFunctionType.Relu, bias=sb_b[l][:])
                if dnt in SRC_NTYPES:
                    stage_ag_window(l + 1, dnt, w * P, cols)
            else:
                fin = wpool.tile([P, P], mybir.dt.float32, tag="fin")
                nc.scalar.activation(
                    fin[:od, :cols], pb[:od, :cols],
                    mybir.ActivationFunctionType.Identity, bias=sb_b[l][:])
                pt = pst.tile([P, P], mybir.dt.float32, tag="tp")
                nc.tensor.transpose(pt[:cols, :od], fin[:od, :cols],
                                    identity[:od, :od])
                stg = wpool.tile([P, DOUT], mybir.dt.float32, tag="ostg")
                nc.vector.tensor_copy(stg[:cols, :], pt[:cols, :od])
                nc.sync.dma_start(out_par[cs:ce, :], stg[:cols, :])

        def emit_phase(l, dnt, tags):
            for w in range(nw[dnt]):
                emit_window(l, dnt, tags, w)

        # ---- program ----
        # Each window's activation stages its next-layer AllGather input;
        # the collective for an ntype is dispatched right after that ntype's
        # windows, so its wire time overlaps the remaining phases' gathers.
        emit_embedding("drug")
        emit_ag(0, "drug")
        emit_embedding("gene")
        emit_ag(0, "gene")
        emit_embedding("disease")
        for l in range(3):
            emit_phase(l, *PHASES[0])
            if l < 2:
                emit_ag_collective(l + 1, "drug")
            emit_phase(l, *PHASES[1])
            if l < 2:
                emit_ag_collective(l + 1, "gene")
            emit_phase(l, *PHASES[2])

    nc.compile()
    return nc


# ---------------------------------------------------------------------------
# entry point
# ---------------------------------------------------------------------------

def _install_ntff_hook():
    if "antenv.axon_hooks" in sys.modules:
        return
    mod = types.ModuleType("antenv.axon_hooks")
    mod._hook = None
    mod.set_axon_ntff_profile_hook = lambda h: setattr(mod, "_hook", h)
    mod.get_axon_ntff_profile_hook = lambda: mod._hook
    sys.modules["antenv.axon_hooks"] = mod
    try:
        import antenv
        antenv.axon_hooks = mod
        from trn_agent_boot.trn_boot import _ntff_profile_via_ctypes
        hook = _ntff_profile_via_ctypes("/opt/axon/libaxon_pjrt.so")
        if hook is not None:
            mod.set_axon_ntff_profile_hook(hook)
    except Exception:
        pass


def run(inputs, cfg=CFG, trace=False, tmpdir=None):
    S, percore = preprocess(cfg, inputs)
    nc = build(S)
    _install_ntff_hook()
    from concourse import bass_utils
    bass_utils.upload_artifacts = lambda d: d
    res = bass_utils.run_bass_kernel_spmd(
        nc, percore, list(range(cfg["NCORE"])), trace=trace, tmpdir=tmpdir,
        trace_cores=[0] if trace else None)
    ncore = cfg["NCORE"]
    shard = {nt: cfg["N"][nt] // ncore for nt in NTYPES}
    outs = []
    o = 0
    for nt in NTYPES:
        parts = [res.results[c]["out"][o:o + shard[nt]] for c in range(ncore)]
        outs.append(np.concatenate(parts, 0))
        o += shard[nt]
    full = np.concatenate(outs, 0).astype(np.float32)
    run.last_exec_time_ns = res.exec_time_ns
    return full


def kernel(**inputs):
    return run(inputs)

